# revision 1
# baseline (speedup 1.0000x reference)
"""Trainium2 Bass kernel for nn_MoEAdaptorLayer (whiten -> causal MHA -> proj
-> noisy-top-k gating (eval) -> 8 dense experts -> gated mixture * expert sum).

Sharding: data-parallel over batch. 64 batches -> 8 per core, params replicated.
Each core processes its 8 batches (2400 tokens) end to end; host concatenates.

Host-side preprocessing folds the whiten linear into the QKV projections
(q = (x - b) @ whW.T @ Wq.T = x @ (Wq @ whW).T + const), transposes weights
into the layouts the matmuls need, and casts to bf16. All matmuls run in bf16
with fp32 PSUM accumulation.

Device dataflow (all per core, activations kept transposed [feat, tok] until
the expert stage):
  qkv:   qkT/vT[192, 2400] = WqkvT.T @ xT        (xT: [768, 2400] bf16)
  attn:  per (batch, head): scores both orientations (K=16 matmuls);
         [t,s]-pass -> exp -> masked-accum => Z (softmax denominator);
         [s,t]-pass -> exp -> causal zero (affine_select) => FT;
         attu[t, hk] = FT.T @ v (psum accum over s-chunks); divide by Z
         (per-partition scalars); PE-transpose -> attnT[64, 2400]
  proj:  xaT[300, 2400] = projWT.T @ attnT (+bias)
  gate:  logits[t, 8] = xaT_chunk.T @ w_gate; top-4 threshold via 4 rounds of
         reduce_max + mask-out; sparse softmax via exp * sel / sum
  exp:   per expert e: hT[600, 2400] = relu(W1T.T @ xaT + b1);
         eo[t, 300] = hT_chunk.T @ W2T (psum); G += gprob_e * eo (one fused
         scalar_tensor_tensor), S += eo
  out:   out[t, 300] = G * S  -> DRAM
"""

from contextlib import ExitStack

import numpy as np
import ml_dtypes

import concourse.bass as bass
import concourse.tile as tile
import concourse.mybir as mybir
from concourse.bass_utils import run_bass_kernel_spmd
from concourse.masks import make_identity

BF16 = mybir.dt.bfloat16
F32 = mybir.dt.float32
F32R = mybir.dt.float32r
AX = mybir.AxisListType
OP = mybir.AluOpType
AF = mybir.ActivationFunctionType
nbf16 = ml_dtypes.bfloat16

B, T, DIN, D, E, H, HS = 64, 300, 768, 300, 8, 4, 16
NCORES = 8
BL = B // NCORES          # 8 batches per core
NTOK = BL * T             # 2400 tokens per core
DHK = H * HS              # 64
DH = 2 * D                # 600
TOPK = E // 2

K768 = [(i * 128, 128) for i in range(6)]
DC = [(0, 128), (128, 128), (256, 44)]          # 300 = 128+128+44
HC = [(i * 120, 120) for i in range(5)]         # 600 = 5*120
TB = [(0, 128), (128, 128), (256, 44)]          # 300 tokens per batch
TOK = [(i * 128, 128) for i in range(18)] + [(2304, 96)]   # 2400 tokens
N5 = [(i * 480, 480) for i in range(5)]         # 2400 free-dim chunks
NEG = -1.0e30

_MAX_DRAIN_WAITS = 1
_WAIT_LIMIT = 1


def _split_waits(nc):
    """Walrus in this build caps sync waits per instruction; hoist excess
    waits onto same-engine NOPs inserted just before the instruction."""
    n = 0
    for f in nc.m.functions:
        for blk in f.blocks:
            insts = blk.instructions
            out = []
            changed = False
            for inst in insts:
                si = inst.sync_info
                waits = list(si.on_wait or []) if si is not None else []
                if len(waits) > _WAIT_LIMIT:
                    head, tail = waits[:-_WAIT_LIMIT], waits[-_WAIT_LIMIT:]
                    for i in range(0, len(head), _WAIT_LIMIT):
                        n += 1
                        nop = mybir.InstNoOp(name=f"waitnop{n}", ins=[], outs=[])
                        nop.engine = inst.engine
                        nop.sync_info = mybir.SyncInfo(
                            on_wait=head[i : i + _WAIT_LIMIT], on_update=[]
                        )
                        out.append(nop)
                    si.on_wait = tail
                    inst.sync_info = si
                    changed = True
                out.append(inst)
            if changed:
                blk.instructions = out


def _install_ldw_opt():
    """Enable walrus's redundant-LDWEIGHTS elision (off by default in
    bass_utils): rewrite the flag in the walrus argv at compile time."""
    import concourse.bass_utils as bu
    if getattr(bu, "_ldw_patched", False):
        return
    orig = bu.run_command

    def patched(argv, **kw):
        return orig(argv, **kw)

    bu.run_command = patched
    bu._ldw_patched = True


def _install_drain_patch():
    """This walrus build rejects CTRL instructions with more than a few sync
    waits; Tile's tail drain waits on every engine/DMA semaphore at once.
    Split the waits across a chain of single-wait drains."""
    if getattr(tile.TileContext, "_drain_patched", False):
        return

    def _patched(self, tick_clock, wait_clock):
        nc = self.nc
        drain_inst = nc.sync.drain()
        wait_clock.add_sem_waits(
            drain_inst.ins, tile.ScopedClock({None: tick_clock.global_clock})
        )
        ri = drain_inst.ins
        si = ri.sync_info
        waits = list(si.on_wait or []) if si is not None else []
        if len(waits) > _MAX_DRAIN_WAITS:
            si.on_wait = waits[:_MAX_DRAIN_WAITS]
            ri.sync_info = si
            for i in range(_MAX_DRAIN_WAITS, len(waits), _MAX_DRAIN_WAITS):
                d2 = nc.sync.drain()
                d2.ins.sync_info = mybir.SyncInfo(
                    on_wait=waits[i : i + _MAX_DRAIN_WAITS], on_update=[]
                )
        nc.all_engine_barrier()
        assert self.sems is not None
        popped = nc._tile_sem_poison_stack.pop()
        assert popped is self._sem_poison
        nc.clear_and_free_semaphores(list(self.sems.allocated().values()))
        nc.all_engine_barrier()

    tile.TileContext._drain_and_barrier = _patched
    tile.TileContext._drain_patched = True


def _build_module(b2_nonzero: bool, debug: bool = False, max_phase: int = 9):
    _install_drain_patch()
    _install_ldw_opt()
    nc = bass.Bass("TRN2", target_bir_lowering=False, debug=False)

    # ---- DRAM I/O ----
    xt_d = nc.dram_tensor("xt", [BL, DIN, T], F32R, kind="ExternalInput")
    wqkv_d = nc.dram_tensor("wqkv", [DIN, 320], F32R, kind="ExternalInput")
    qkvb_d = nc.dram_tensor("qkvb", [128, 3], F32, kind="ExternalInput")
    projw_d = nc.dram_tensor("projw", [DHK, D], F32R, kind="ExternalInput")
    projb_d = nc.dram_tensor("projb", [D, 1], F32, kind="ExternalInput")
    wg_d = nc.dram_tensor("wg", [D, E], F32R, kind="ExternalInput")
    tri_d = nc.dram_tensor("tri", [T, T], F32, kind="ExternalInput")
    w1_d = nc.dram_tensor("w1t", [E, D, DH], BF16, kind="ExternalInput")
    w2_d = nc.dram_tensor("w2t", [E, DH, D], BF16, kind="ExternalInput")
    b1_d = nc.dram_tensor("b1s", [120, E * 5], F32, kind="ExternalInput")
    b2_d = nc.dram_tensor("b2s", [E, 1, D], BF16, kind="ExternalInput")
    idm_d = nc.dram_tensor("idm", [128, 128], F32R, kind="ExternalInput")
    out_d = nc.dram_tensor("out", [NTOK, D], F32, kind="ExternalOutput")
    if debug:
        dbg_q = nc.dram_tensor("dbg_q", [128, NTOK], F32, kind="ExternalOutput")
        dbg_k = nc.dram_tensor("dbg_k", [128, NTOK], F32, kind="ExternalOutput")
        dbg_v = nc.dram_tensor("dbg_v", [DHK, NTOK], F32, kind="ExternalOutput")
        dbg_at = nc.dram_tensor("dbg_at", [DHK, NTOK], F32, kind="ExternalOutput")
        dbg_xa = nc.dram_tensor("dbg_xa", [D, NTOK], F32, kind="ExternalOutput")
        dbg_gp = nc.dram_tensor("dbg_gp", [128, len(TOK) * E], F32, kind="ExternalOutput")
        dbg_xt = nc.dram_tensor("dbg_xt", [128, BL, T], F32, kind="ExternalOutput")

    with tile.TileContext(nc) as tc:
        with (
            tc.tile_pool(name="const", bufs=1) as cpool,
            tc.tile_pool(name="big", bufs=1) as bpool,
        ):
            # ---- persistent constants ----
            ident = cpool.tile([128, 128], F32R)
            nc.sync.dma_start(ident[:, :], idm_d[:, :])
            wqkv_s = []
            for kc, (k0, ks) in enumerate(K768):
                t = cpool.tile([128, 320], F32R, tag=f"wqkv{kc}")
                nc.sync.dma_start(t[:, :], wqkv_d[k0 : k0 + ks, :])
                wqkv_s.append(t)
            qkvb_s = cpool.tile([128, 3], F32, tag="qkvbs")
            nc.sync.dma_start(qkvb_s[:, :], qkvb_d[:, :])
            projw_s = cpool.tile([DHK, D], F32R, tag="projw")
            nc.sync.dma_start(projw_s[:, :], projw_d[:, :])
            projb_s, wg_s, tri_s = [], [], []
            for dc, (d0, ds) in enumerate(DC):
                t = cpool.tile([ds, 1], F32, tag=f"projb{dc}")
                nc.sync.dma_start(t[:, :], projb_d[d0 : d0 + ds, :])
                projb_s.append(t)
                t = cpool.tile([ds, E], F32R, tag=f"wg{dc}")
                nc.sync.dma_start(t[:, :], wg_d[d0 : d0 + ds, :])
                wg_s.append(t)
            for tcb, (t0, ts_) in enumerate(TB):
                t = cpool.tile([ts_, T], F32, tag=f"tri{tcb}")
                nc.sync.dma_start(t[:, :], tri_d[t0 : t0 + ts_, :])
                tri_s.append(t)
            b1_s = cpool.tile([120, E * 5], F32, tag="b1s")
            nc.sync.dma_start(b1_s[:, :], b1_d[:, :])

            # prefetch experts 0-3 weights up front on the gpsimd DMA queue
            w1_all, w2_all, b2_all = [], [], []
            for e in range(E // 2):
                w1t = []
                for dc, (d0, ds) in enumerate(DC):
                    t = bpool.tile([ds, DH], BF16, tag=f"w1_{e}_{dc}", name=f"w1_{e}_{dc}")
                    nc.gpsimd.dma_start(t[:, :], w1_d[e, d0 : d0 + ds, :])
                    w1t.append(t)
                w1_all.append(w1t)
                w2t = []
                for mc, (h0, hs) in enumerate(HC):
                    t = bpool.tile([120, D], BF16, tag=f"w2_{e}_{mc}", name=f"w2_{e}_{mc}")
                    nc.gpsimd.dma_start(t[:, :], w2_d[e, h0 : h0 + hs, :])
                    w2t.append(t)
                w2_all.append(w2t)
                if b2_nonzero:
                    t = bpool.tile([1, D], BF16, tag=f"b2row{e}", name=f"b2row{e}")
                    nc.gpsimd.dma_start(t[:, :], b2_d[e, :, :])
                    b2_all.append(t)

            # ---- activations ----
            # mid pool: lifetime = qkv..proj; freed before the expert phase
            _mid = ExitStack()
            mpool = _mid.enter_context(tc.tile_pool(name="mid", bufs=1))
            # heads striped at 32-partition bases: tile *a holds heads 0-1 at
            # rows 0:16 / 32:48, tile *b holds heads 2-3 (matmul base paritions
            # must be 0/32/64)
            qTs = mpool.tile([128, NTOK], F32R, tag="qTs")
            kTs = mpool.tile([128, NTOK], F32R, tag="kTs")
            qTb = mpool.tile([DHK, NTOK], F32R, tag="qTb")
            kTb = mpool.tile([DHK, NTOK], F32R, tag="kTb")
            vT = mpool.tile([DHK, NTOK], F32R, tag="vT")
            v_s = [
                mpool.tile([128, BL, DHK], F32R, tag=f"v{sc}", name=f"v{sc}")
                for sc in range(3)
            ]
            attnT = mpool.tile([DHK, NTOK], F32R, tag="attnT")
            xaT = [
                bpool.tile([ds, NTOK], F32R, tag=f"xaT{dc}", name=f"xaT{dc}")
                for dc, (d0, ds) in enumerate(DC)
            ]
            xaTb = [
                bpool.tile([ds, NTOK], BF16, tag=f"xaTb{dc}", name=f"xaTb{dc}")
                for dc, (d0, ds) in enumerate(DC)
            ]

            # ================= phase 1: qkv =================
            with (
                tc.tile_pool(name="xt", bufs=1) as xpool,
                tc.tile_pool(name="ps1", bufs=1, space="PSUM") as ps1,
            ):
                xt_s = [
                    xpool.tile([128, BL, T], F32R, tag=f"xt{kc}", name=f"xt{kc}")
                    for kc in range(6)
                ]
                for b in range(BL):
                    for kc, (k0, ks) in enumerate(K768):
                        nc.sync.dma_start(
                            xt_s[kc][:, b, :], xt_d[b, k0 : k0 + ks, :]
                        )
                specs = ((qTs, 128, 0), (kTs, 128, 128), (vT, DHK, 256))
                for wave in range(2):
                    bs = range(wave * 4, wave * 4 + 4)
                    for mi, (dst, mw, c0) in enumerate(specs):
                        ps = [
                            ps1.tile([128, T], F32, tag=f"qkvp{n}", name=f"qkvp{n}")
                            for n in range(4)
                        ]
                        for kc in range(6):
                            for n, b in enumerate(bs):
                                nc.tensor.matmul(
                                    ps[n][:mw, :],
                                    wqkv_s[kc][:, c0 : c0 + mw],
                                    xt_s[kc][:, b, :],
                                    start=(kc == 0), stop=(kc == 5),
                                )
                        for n, b in enumerate(bs):
                            nc.scalar.activation(
                                dst[:, b * T : (b + 1) * T], ps[n][:mw, :], AF.Identity,
                                bias=qkvb_s[:mw, mi : mi + 1], scale=1.0,
                            )
                # move heads 2-3 down to base 0/32 (DMA shifts partitions)
                nc.sync.dma_start(qTb[:, :], qTs[64:128, :])
                nc.sync.dma_start(kTb[:, :], kTs[64:128, :])
                if debug:
                    nc.sync.dma_start(dbg_xt[:, :, :], xt_s[0][:, :, :])
                    nc.sync.dma_start(dbg_q[:, :], qTs[:, :])

            # ================= phase 2: attention =================
            if max_phase >= 2:
             with (
                tc.tile_pool(name="att", bufs=2) as apool,
                tc.tile_pool(name="ps2", bufs=2, space="PSUM") as ps2,
            ):
                for b in range(BL):
                    base = b * T
                    # v = vT.T per s-chunk (all heads at once)
                    for sc, (s0, ss) in enumerate(TB):
                        vtr = ps2.tile([128, DHK], F32R, tag="tpse")
                        nc.tensor.transpose(
                            vtr[:ss, :], vT[:, base + s0 : base + s0 + ss],
                            ident[0:DHK, 0:DHK],
                        )
                        nc.scalar.copy(v_s[sc][0:ss, b, :], vtr[:ss, :])
                    ft = {}
                    rz = {}
                    for h in range(H):
                        qt = qTs if h < 2 else qTb
                        kt = kTs if h < 2 else kTb
                        q0 = k0 = (h % 2) * 32
                        # Z-pass: scores [t, s], exp, masked accumulate
                        z = apool.tile([128, 4], F32, tag=f"z{h}")
                        for tcb, (t0, ts_) in enumerate(TB):
                            send = t0 + ts_
                            wei = ps2.tile([128, T], F32, tag="wei")
                            nc.tensor.matmul(
                                wei[:ts_, :send],
                                qt[q0 : q0 + HS, base + t0 : base + t0 + ts_],
                                kt[k0 : k0 + HS, base : base + send],
                                start=True, stop=True,
                            )
                            escr = apool.tile([128, T], F32, tag="escr")
                            nc.scalar.activation(escr[:ts_, :send], wei[:ts_, :send], AF.Exp)
                            zjunk = apool.tile([128, T], F32, tag="zjunk")
                            nc.vector.scalar_tensor_tensor(
                                zjunk[:ts_, :send], escr[:ts_, :send], 1.0,
                                tri_s[tcb][:ts_, :send],
                                op0=OP.mult, op1=OP.mult,
                                accum_out=z[:ts_, tcb : tcb + 1],
                            )
                        rzt = apool.tile([128, 4], F32, tag=f"rz{h}")
                        nc.vector.reciprocal(rzt[:, 0:3], z[:, 0:3])
                        rz[h] = rzt
                        # FT-pass: scores [s, t], exp, causal zero on diag block
                        for sc, (s0, ss) in enumerate(TB):
                            tlen = T - s0
                            weiT = ps2.tile([128, T], F32, tag="weiT")
                            nc.tensor.matmul(
                                weiT[:ss, :tlen],
                                kt[k0 : k0 + HS, base + s0 : base + s0 + ss],
                                qt[q0 : q0 + HS, base + s0 : base + T],
                                start=True, stop=True,
                            )
                            f = apool.tile([128, T], F32R, tag=f"ft{h}{sc}")
                            nc.scalar.activation(f[:ss, :tlen], weiT[:ss, :tlen], AF.Exp)
                            nc.gpsimd.affine_select(
                                out=f[:ss, :ss], in_=f[:ss, :ss],
                                compare_op=OP.is_ge, fill=0.0,
                                base=0, channel_multiplier=-1, pattern=[[1, ss]],
                            )
                            ft[(h, sc)] = f
                    # AV + divide + transpose back
                    for tcb, (t0, ts_) in enumerate(TB):
                        attu = ps2.tile([128, DHK], F32, tag="attu")
                        for h in range(H):
                            for sc in range(tcb + 1):
                                s0, ss = TB[sc]
                                nc.tensor.matmul(
                                    attu[:ts_, h * HS : (h + 1) * HS],
                                    ft[(h, sc)][:ss, t0 - s0 : t0 - s0 + ts_],
                                    v_s[sc][0:ss, b, h * HS : (h + 1) * HS],
                                    start=(sc == 0), stop=(sc == tcb),
                                )
                        ascr = apool.tile([128, DHK], F32R, tag="ascr")
                        for h in range(H):
                            nc.vector.tensor_scalar_mul(
                                ascr[:ts_, h * HS : (h + 1) * HS],
                                attu[:ts_, h * HS : (h + 1) * HS],
                                rz[h][:ts_, tcb : tcb + 1],
                            )
                        atp = ps2.tile([DHK, 128], F32R, tag="tpse")
                        nc.tensor.transpose(
                            atp[:, :ts_], ascr[:ts_, :], ident[:ts_, :ts_]
                        )
                        nc.scalar.copy(attnT[:, base + t0 : base + t0 + ts_], atp[:, :ts_])

            # ================= phase 3: proj =================
            if max_phase >= 3:
             with tc.tile_pool(name="ps3", bufs=2, space="PSUM") as ps3:
                for dc, (d0, ds) in enumerate(DC):
                    for n0, ns in N5:
                        xap = ps3.tile([128, 480], F32, tag="xap")
                        nc.tensor.matmul(
                            xap[:ds, :ns], projw_s[:, d0 : d0 + ds],
                            attnT[:, n0 : n0 + ns], start=True, stop=True,
                        )
                        nc.scalar.activation(
                            xaT[dc][:, n0 : n0 + ns], xap[:ds, :ns], AF.Identity,
                            bias=projb_s[dc][:, :], scale=1.0,
                        )
                        nc.gpsimd.tensor_copy(
                            xaTb[dc][:, n0 : n0 + ns], xaT[dc][:, n0 : n0 + ns]
                        )

            if debug:
                nc.sync.dma_start(dbg_k[:, :], kTs[:, :])
                if max_phase >= 2:
                    nc.sync.dma_start(dbg_v[:, :], vT[:, :])
                    nc.sync.dma_start(dbg_at[:, :], attnT[:, :])
                if max_phase >= 3:
                    for dc, (d0, ds) in enumerate(DC):
                        nc.sync.dma_start(dbg_xa[d0 : d0 + ds, :], xaT[dc][:, :])

            _mid.close()
            _late = ExitStack()
            lpool = _late.enter_context(tc.tile_pool(name="late", bufs=1))
            gp = lpool.tile([128, len(TOK) * E], F32, tag="gp")
            G = lpool.tile([128, len(TOK) * D], F32, tag="G")
            S = lpool.tile([128, len(TOK) * D], F32, tag="S")

            # ================= phase 4: gating (batched wide ops) =========
            if max_phase >= 4:
             with (
                tc.tile_pool(name="gat", bufs=1) as gpool,
                tc.tile_pool(name="ps4", bufs=1, space="PSUM") as ps4,
             ):
                NT_ = len(TOK)
                lp = ps4.tile([128, NT_ * E], F32, tag="lp")
                for ti, (t0, ts_) in enumerate(TOK):
                    for dc, (d0, ds) in enumerate(DC):
                        nc.tensor.matmul(
                            lp[:ts_, ti * E : (ti + 1) * E],
                            xaT[dc][:, t0 : t0 + ts_], wg_s[dc][:, :],
                            start=(dc == 0), stop=(dc == 2),
                        )
                L3 = gpool.tile([128, NT_, E], F32, tag="L3")
                nc.scalar.copy(
                    L3[:, :, :], lp[:, :].rearrange("p (t e) -> p t e", e=E)
                )
                work = gpool.tile([128, NT_, E], F32, tag="work")
                nc.vector.tensor_copy(work[:, :, :], L3[:, :, :])
                m = gpool.tile([128, NT_, 4], F32, tag="m")
                eqm = gpool.tile([128, NT_, E], F32, tag="eqm")
                for r in range(TOPK):
                    nc.vector.tensor_reduce(
                        m[:, :, r : r + 1], work[:, :, :], axis=AX.X, op=OP.max,
                        opt_input=False, opt_output=False,
                    )
                    if r < TOPK - 1:
                        nc.vector.tensor_tensor(
                            eqm[:, :, :], work[:, :, :],
                            m[:, :, r : r + 1].broadcast_to((128, NT_, E)),
                            op=OP.is_equal,
                        )
                        nc.vector.scalar_tensor_tensor(
                            work[:, :, :], eqm[:, :, :], NEG, work[:, :, :],
                            op0=OP.mult, op1=OP.add,
                        )
                sel = gpool.tile([128, NT_, E], F32, tag="sel")
                nc.vector.tensor_tensor(
                    sel[:, :, :], L3[:, :, :],
                    m[:, :, 3:4].broadcast_to((128, NT_, E)), op=OP.is_ge,
                )
                el = gpool.tile([128, NT_, E], F32, tag="el")
                nc.scalar.activation(el[:, :, :], L3[:, :, :], AF.Exp)
                elm = gpool.tile([128, NT_, E], F32, tag="elm")
                nc.vector.tensor_tensor(elm[:, :, :], el[:, :, :], sel[:, :, :], op=OP.mult)
                zg = gpool.tile([128, NT_, 1], F32, tag="zg")
                nc.vector.tensor_reduce(
                    zg[:, :, :], elm[:, :, :], axis=AX.X, op=OP.add,
                    opt_input=False, opt_output=False,
                )
                rzg = gpool.tile([128, NT_, 1], F32, tag="rzg")
                nc.vector.reciprocal(rzg[:, :, :], zg[:, :, :])
                nc.vector.tensor_tensor(
                    gp[:, :].rearrange("p (t e) -> p t e", e=E), elm[:, :, :],
                    rzg[:, :, :].broadcast_to((128, NT_, E)),
                    op=OP.mult,
                )
            if debug and max_phase >= 4:
                nc.sync.dma_start(dbg_gp[:, :], gp[:, :])

            # ================= phase 5: experts =================
            if max_phase >= 5:
             with (
                tc.tile_pool(name="outp", bufs=4) as opool,
                tc.tile_pool(name="wexp", bufs=1) as wpool,
                tc.tile_pool(name="ht", bufs=2) as hpool,
                tc.tile_pool(name="ps5", bufs=1, space="PSUM") as ps5h,
                tc.tile_pool(name="ps5b", bufs=3, space="PSUM") as ps5e,
            ):
                for e in range(E // 2, E):
                    w1t = []
                    for dc, (d0, ds) in enumerate(DC):
                        t = wpool.tile([ds, DH], BF16, tag=f"w1_{e}_{dc}", name=f"w1_{e}_{dc}")
                        nc.gpsimd.dma_start(t[:, :], w1_d[e, d0 : d0 + ds, :])
                        w1t.append(t)
                    w1_all.append(w1t)
                    w2t = []
                    for mc, (h0, hs) in enumerate(HC):
                        t = wpool.tile([120, D], BF16, tag=f"w2_{e}_{mc}", name=f"w2_{e}_{mc}")
                        nc.gpsimd.dma_start(t[:, :], w2_d[e, h0 : h0 + hs, :])
                        w2t.append(t)
                    w2_all.append(w2t)
                    if b2_nonzero:
                        t = wpool.tile([1, D], BF16, tag=f"b2row{e}", name=f"b2row{e}")
                        nc.gpsimd.dma_start(t[:, :], b2_d[e, :, :])
                        b2_all.append(t)
                ones = None
                if b2_nonzero:
                    ones = cpool.tile([1, NTOK], BF16, tag="ones")
                    nc.gpsimd.memset(ones[:, :], 1.0)
                for e in range(E):
                    w1t = w1_all[e]
                    w2t = w2_all[e]
                    b2row = b2_all[e] if b2_nonzero else None
                    ht = []
                    for mc, (h0, hs) in enumerate(HC):
                        t = hpool.tile([120, NTOK], BF16, tag=f"ht{mc}")
                        ht.append(t)
                        hps = [
                            ps5h.tile([120, 480], F32, tag=f"hp{n}", name=f"hp{n}")
                            for n in range(5)
                        ]
                        for dc in range(3):
                            for n, (n0, ns) in enumerate(N5):
                                nc.tensor.matmul(
                                    hps[n][:, :ns], w1t[dc][:, h0 : h0 + hs],
                                    xaTb[dc][:, n0 : n0 + ns],
                                    start=(dc == 0), stop=(dc == 2),
                                )
                        for n, (n0, ns) in enumerate(N5):
                            nc.scalar.activation(
                                t[:, n0 : n0 + ns], hps[n][:, :ns], AF.Relu,
                                bias=b1_s[:, e * 5 + mc : e * 5 + mc + 1], scale=1.0,
                            )
                    for ti, (t0, ts_) in enumerate(TOK):
                        eo = ps5e.tile([128, D], F32, tag="eo")
                        for mc in range(5):
                            nc.tensor.matmul(
                                eo[:ts_, :], ht[mc][:, t0 : t0 + ts_], w2t[mc][:, :],
                                start=(mc == 0), stop=(mc == 4 and not b2_nonzero),
                            )
                        if b2_nonzero:
                            nc.tensor.matmul(
                                eo[:ts_, :], ones[:, t0 : t0 + ts_], b2row[:, :],
                                start=False, stop=True,
                            )
                        gsc = gp[:ts_, ti * E + e : ti * E + e + 1]
                        gsl = G[:ts_, ti * D : (ti + 1) * D]
                        ssl = S[:ts_, ti * D : (ti + 1) * D]
                        if e == 0:
                            nc.vector.tensor_scalar_mul(gsl, eo[:ts_, :], gsc)
                            nc.scalar.copy(ssl, eo[:ts_, :])
                        else:
                            nc.vector.scalar_tensor_tensor(
                                gsl, eo[:ts_, :], gsc, gsl, op0=OP.mult, op1=OP.add
                            )
                            nc.vector.tensor_tensor(ssl, eo[:ts_, :], ssl, op=OP.add)
                        if e == E - 1:
                            o = opool.tile([128, D], F32, tag="o")
                            nc.vector.tensor_tensor(o[:ts_, :], gsl, ssl, op=OP.mult)
                            nc.sync.dma_start(out_d[t0 : t0 + ts_, :], o[:ts_, :])

            _late.close()

    _split_waits(nc)
    return nc


_CACHE = {}
LAST_RESULT = None


def _get_module(b2_nonzero: bool):
    key = b2_nonzero
    if key not in _CACHE:
        _CACHE[key] = _build_module(b2_nonzero)
    return _CACHE[key]


def kernel(x, wh_bias, wh_W, Wq, Wk, Wv, proj_W, proj_b,
           exp_W1, exp_b1, exp_W2, exp_b2, w_gate):
    global LAST_RESULT
    x = np.asarray(x, np.float32)
    wh_bias = np.asarray(wh_bias, np.float32)
    wh_W = np.asarray(wh_W, np.float32)
    Wq, Wk, Wv = (np.asarray(w, np.float32) for w in (Wq, Wk, Wv))
    proj_W = np.asarray(proj_W, np.float32)
    proj_b = np.asarray(proj_b, np.float32)
    exp_W1 = np.asarray(exp_W1, np.float32)
    exp_b1 = np.asarray(exp_b1, np.float32)
    exp_W2 = np.asarray(exp_W2, np.float32)
    exp_b2 = np.asarray(exp_b2, np.float32)
    w_gate = np.asarray(w_gate, np.float32)

    # fold whiten into qkv; fold attention scale into q
    scale = float(D) ** -0.5
    Wqf = (Wq.reshape(DHK, D) @ wh_W) * scale          # [64, 768]
    Wkf = Wk.reshape(DHK, D) @ wh_W
    Wvf = Wv.reshape(DHK, D) @ wh_W
    def stripe(w):                                     # [64, 768] -> [128, 768]
        out = np.zeros((128, DIN), np.float32)
        for h in range(H):
            out[h * 32 : h * 32 + HS] = w[h * HS : (h + 1) * HS]
        return out

    def stripe_b(v):                                   # [64] -> [128]
        out = np.zeros(128, np.float32)
        for h in range(H):
            out[h * 32 : h * 32 + HS] = v[h * HS : (h + 1) * HS]
        return out

    wqkv = np.concatenate([stripe(Wqf), stripe(Wkf), Wvf], 0)   # [320, 768]
    bq = -(Wqf @ wh_bias)
    bk = -(Wkf @ wh_bias)
    bv = -(Wvf @ wh_bias)
    qkvb = np.stack([stripe_b(bq), stripe_b(bk), np.pad(bv, (0, 64))], 1)  # [128, 3]

    tri = np.tril(np.ones((T, T), np.float32))
    b1s = np.ascontiguousarray(
        exp_b1.reshape(E, 5, 120).transpose(2, 0, 1).reshape(120, E * 5)
    )
    b2_nonzero = bool(np.any(exp_b2))

    common = {
        "wqkv": np.ascontiguousarray(wqkv.T).astype(np.float32),
        "qkvb": np.ascontiguousarray(qkvb).astype(np.float32),
        "projw": np.ascontiguousarray(proj_W.T).astype(np.float32),
        "projb": np.ascontiguousarray(proj_b[:, None]).astype(np.float32),
        "wg": np.ascontiguousarray(w_gate).astype(np.float32),
        "tri": tri.astype(np.float32),
        "idm": np.eye(128, dtype=np.float32),
        "w1t": np.ascontiguousarray(exp_W1.transpose(0, 2, 1)).astype(nbf16),
        "w2t": np.ascontiguousarray(exp_W2.transpose(0, 2, 1)).astype(nbf16),
        "b1s": b1s.astype(np.float32),
        "b2s": np.ascontiguousarray(exp_b2[:, None, :]).astype(nbf16),
    }
    in_maps = []
    for c in range(NCORES):
        xc = x[c * BL : (c + 1) * BL]                  # [8, 300, 768]
        xt = np.ascontiguousarray(xc.transpose(0, 2, 1)).astype(np.float32)
        in_maps.append({**common, "xt": xt})

    nc = _get_module(b2_nonzero)
    for alloc in nc.m.functions[0].allocations:
        if isinstance(alloc, mybir.MemoryLocationSet) and alloc.kind == "ExternalInput":
            nm = alloc.memorylocations[0].name
            if nm not in in_maps[0]:
                continue  # partition_id etc., supplied by the runner
            got = in_maps[0][nm]
            assert tuple(got.shape) == tuple(alloc.tensor_shape), (
                nm, got.shape, alloc.tensor_shape)
            assert got.dtype == mybir.dt.np(alloc.dtype), (nm, got.dtype)
    res = run_bass_kernel_spmd(nc, in_maps, core_ids=list(range(NCORES)))
    LAST_RESULT = res
    out = np.stack([r["out"] for r in res.results])    # [8, 2400, 300]
    return out.reshape(B, T, D)



# revision 23
# speedup vs baseline: 1.1459x; 1.1459x over previous
"""Trainium2 Bass kernel for nn_MoEAdaptorLayer (whiten -> causal MHA -> proj
-> noisy-top-k gating (eval) -> 8 dense experts -> gated mixture * expert sum).

Sharding: data-parallel over batch. 64 batches -> 8 per core, params replicated.

v2 redesign vs baseline:
- fp16 everywhere on the matmul path (fp32 PSUM accumulation); halves DMA and
  removes the fp32r small-free-dim PE penalties.
- qkv: x shipped as one [768, 2400] fp16 tensor per core (6 big DMAs), matmuls
  in [128, 480] chunks.
- attention: single-orientation flash-style pass. Scores computed [s, t] only;
  exp on scalar; causal mask by multiplying the diagonal blocks with a
  triangular constant (split vector/gpsimd). AV and the softmax denominator
  come from ONE matmul per (head, s-chunk): stationary [v_h | ones] (32 cols)
  so the psum holds [av_h (16 rows) | Z_h replicated (16 rows)] per 32-stripe.
  One reciprocal + one fused min-mult STT per batch produce attnT directly in
  head-striped [128, tok] layout (no Z-pass, no output transposes).
- proj: stationary is zero-padded to the 32-stripe layout; 15 fp16 matmuls.
- gating: logits computed transposed ([8, tok], weight-stationary, 15 matmuls
  instead of 57 LDW-bound ones), PE-transposed back in 19 tiny transposes.
- experts: h chunked {128,128,128,128,88} (M=128-aligned LDWEIGHTS is ~2x
  faster than M=120), fp16 weights/activations.
"""

from contextlib import ExitStack

import numpy as np

import concourse.bass as bass
import concourse.tile as tile
import concourse.mybir as mybir
from concourse.bass_utils import run_bass_kernel_spmd

F16 = mybir.dt.float16
F32 = mybir.dt.float32
AX = mybir.AxisListType
OP = mybir.AluOpType
AF = mybir.ActivationFunctionType

B, T, DIN, D, E, H, HS = 64, 300, 768, 300, 8, 4, 16
NCORES = 8
BL = B // NCORES          # 8 batches per core
NTOK = BL * T             # 2400 tokens per core
DHK = H * HS              # 64
DH = 2 * D                # 600
TOPK = E // 2

K768 = [(i * 128, 128) for i in range(6)]
DC = [(0, 128), (128, 128), (256, 44)]          # 300 = 128+128+44
MCH = [(0, 128), (128, 128), (256, 128), (384, 128), (512, 88)]  # 600
TB = [(0, 128), (128, 128), (256, 44)]          # 300 tokens per batch
TOK = [(i * 128, 128) for i in range(18)] + [(2304, 96)]   # 2400 tokens
N5 = [(i * 480, 480) for i in range(5)]         # 2400 free-dim chunks
NEG = -1.0e30
RCLAMP = 16.0   # cap on 1/Z; keeps garbage stripe rows finite in fp16

_MAX_DRAIN_WAITS = 1
_WAIT_LIMIT = 1


def _split_waits(nc):
    """Walrus in this build caps sync waits per instruction; hoist excess
    waits onto same-engine NOPs inserted just before the instruction."""
    n = 0
    for f in nc.m.functions:
        for blk in f.blocks:
            insts = blk.instructions
            out = []
            changed = False
            for inst in insts:
                si = inst.sync_info
                waits = list(si.on_wait or []) if si is not None else []
                if len(waits) > _WAIT_LIMIT:
                    head, tail = waits[:-_WAIT_LIMIT], waits[-_WAIT_LIMIT:]
                    for i in range(0, len(head), _WAIT_LIMIT):
                        n += 1
                        nop = mybir.InstNoOp(name=f"waitnop{n}", ins=[], outs=[])
                        nop.engine = inst.engine
                        nop.sync_info = mybir.SyncInfo(
                            on_wait=head[i : i + _WAIT_LIMIT], on_update=[]
                        )
                        out.append(nop)
                    si.on_wait = tail
                    inst.sync_info = si
                    changed = True
                out.append(inst)
            if changed:
                blk.instructions = out


def _install_drain_patch():
    """This walrus build rejects CTRL instructions with more than a few sync
    waits; Tile's tail drain waits on every engine/DMA semaphore at once.
    Split the waits across a chain of single-wait drains."""
    if getattr(tile.TileContext, "_drain_patched", False):
        return

    def _patched(self, tick_clock, wait_clock):
        nc = self.nc
        drain_inst = nc.sync.drain()
        wait_clock.add_sem_waits(
            drain_inst.ins, tile.ScopedClock({None: tick_clock.global_clock})
        )
        ri = drain_inst.ins
        si = ri.sync_info
        waits = list(si.on_wait or []) if si is not None else []
        if len(waits) > _MAX_DRAIN_WAITS:
            si.on_wait = waits[:_MAX_DRAIN_WAITS]
            ri.sync_info = si
            for i in range(_MAX_DRAIN_WAITS, len(waits), _MAX_DRAIN_WAITS):
                d2 = nc.sync.drain()
                d2.ins.sync_info = mybir.SyncInfo(
                    on_wait=waits[i : i + _MAX_DRAIN_WAITS], on_update=[]
                )
        nc.all_engine_barrier()
        assert self.sems is not None
        popped = nc._tile_sem_poison_stack.pop()
        assert popped is self._sem_poison
        nc.clear_and_free_semaphores(list(self.sems.allocated().values()))
        nc.all_engine_barrier()

    tile.TileContext._drain_and_barrier = _patched
    tile.TileContext._drain_patched = True


def _build_module(b2_nonzero: bool, debug: bool = False, max_phase: int = 9):
    _install_drain_patch()
    nc = bass.Bass("TRN2", target_bir_lowering=False, debug=False)

    # ---- DRAM I/O ----
    xt_d = nc.dram_tensor("xt", [DIN, NTOK], F16, kind="ExternalInput")
    wqkv_d = nc.dram_tensor("wqkv", [DIN, 320], F16, kind="ExternalInput")
    qkvb_d = nc.dram_tensor("qkvb", [128, 3], F32, kind="ExternalInput")
    projw_d = nc.dram_tensor("projw", [2, 64, D], F16, kind="ExternalInput")
    projb_d = nc.dram_tensor("projb", [D, 1], F32, kind="ExternalInput")
    wg_d = nc.dram_tensor("wg", [D, E], F16, kind="ExternalInput")
    mlt_d = nc.dram_tensor("mlt", [128, T], F16, kind="ExternalInput")
    w1_d = nc.dram_tensor("w1t", [E, D, DH], F16, kind="ExternalInput")
    w2_d = nc.dram_tensor("w2t", [E, DH, D], F16, kind="ExternalInput")
    b1_d = nc.dram_tensor("b1s", [128, E * 5], F32, kind="ExternalInput")
    b2_d = nc.dram_tensor("b2s", [E, 1, D], F16, kind="ExternalInput")
    idm_d = nc.dram_tensor("idm", [128, 128], F16, kind="ExternalInput")
    out_d = nc.dram_tensor("out", [NTOK, D], F32, kind="ExternalOutput")
    if debug:
        dbg_q = nc.dram_tensor("dbg_q", [128, NTOK], F32, kind="ExternalOutput")
        dbg_k = nc.dram_tensor("dbg_k", [128, NTOK], F32, kind="ExternalOutput")
        dbg_at = nc.dram_tensor("dbg_at", [128, NTOK], F32, kind="ExternalOutput")
        dbg_xa = nc.dram_tensor("dbg_xa", [D, NTOK], F32, kind="ExternalOutput")
        dbg_gp = nc.dram_tensor("dbg_gp", [128, len(TOK) * E], F32, kind="ExternalOutput")

    with tile.TileContext(nc) as tc:
        with (
            tc.tile_pool(name="const", bufs=1) as cpool,
            tc.tile_pool(name="big", bufs=1) as bpool,
        ):
            # ---- persistent constants ----
            ident = cpool.tile([128, 128], F16)
            nc.sync.dma_start(ident[:, :], idm_d[:, :])
            wqkv_s = []
            for kc, (k0, ks) in enumerate(K768):
                t = cpool.tile([128, 320], F16, tag=f"wqkv{kc}")
                nc.sync.dma_start(t[:, :], wqkv_d[k0 : k0 + ks, :])
                wqkv_s.append(t)
            qkvb_s = cpool.tile([128, 3], F32, tag="qkvbs")
            nc.sync.dma_start(qkvb_s[:, :], qkvb_d[:, :])
            projw_s = []
            for g in range(2):
                t = cpool.tile([64, D], F16, tag=f"projw{g}")
                nc.sync.dma_start(t[:, :], projw_d[g, :, :])
                projw_s.append(t)
            mlt_s = cpool.tile([128, T], F16, tag="mlts")
            nc.sync.dma_start(mlt_s[:, :], mlt_d[:, :])
            projb_s, wg_s = [], []
            for dc, (d0, ds) in enumerate(DC):
                t = cpool.tile([ds, 1], F32, tag=f"projb{dc}")
                nc.sync.dma_start(t[:, :], projb_d[d0 : d0 + ds, :])
                projb_s.append(t)
                t = cpool.tile([ds, E], F16, tag=f"wg{dc}")
                nc.sync.dma_start(t[:, :], wg_d[d0 : d0 + ds, :])
                wg_s.append(t)
            b1_s = cpool.tile([128, E * 5], F32, tag="b1s")
            nc.sync.dma_start(b1_s[:, :], b1_d[:, :])

            # prefetch experts 0-3 weights up front on the gpsimd DMA queue
            w1_all, w2_all, b2_all = [], [], []
            for e in range(E // 2):
                w1t = []
                for dc, (d0, ds) in enumerate(DC):
                    t = bpool.tile([ds, DH], F16, tag=f"w1_{e}_{dc}", name=f"w1_{e}_{dc}")
                    nc.gpsimd.dma_start(t[:, :], w1_d[e, d0 : d0 + ds, :])
                    w1t.append(t)
                w1_all.append(w1t)
                w2t = []
                for mc, (h0, hs) in enumerate(MCH):
                    t = bpool.tile([hs, D], F16, tag=f"w2_{e}_{mc}", name=f"w2_{e}_{mc}")
                    nc.gpsimd.dma_start(t[:, :], w2_d[e, h0 : h0 + hs, :])
                    w2t.append(t)
                w2_all.append(w2t)
                if b2_nonzero:
                    t = bpool.tile([1, D], F16, tag=f"b2row{e}", name=f"b2row{e}")
                    nc.gpsimd.dma_start(t[:, :], b2_d[e, :, :])
                    b2_all.append(t)

            # ---- activations ----
            _mid = ExitStack()
            mpool = _mid.enter_context(tc.tile_pool(name="mid", bufs=1))
            # heads striped at 32-partition bases (rows 32h:32h+16 per head)
            qTs = mpool.tile([128, NTOK], F16, tag="qTs")
            kTs = mpool.tile([128, NTOK], F16, tag="kTs")
            qTb = mpool.tile([DHK, NTOK], F16, tag="qTb")
            kTb = mpool.tile([DHK, NTOK], F16, tag="kTb")
            vT = mpool.tile([DHK, NTOK], F16, tag="vT")
            # v_s[sc]: [ss, b, h, 64] = [v_h(16) | 0(16) | ones(16) | 0(16)];
            # the av/Z layout this produces keeps every vector-op partition
            # base 32-aligned
            v_s = [
                mpool.tile([128, BL, H, 64], F16, tag=f"v{sc}", name=f"v{sc}")
                for sc in range(3)
            ]
            # attnT group tiles [64, NTOK]: rows 0:16 = head 2g, 32:48 =
            # head 2g+1, rest zeros (proj weights zero there)
            attnT = [
                bpool.tile([64, NTOK], F16, tag=f"attnT{g}", name=f"attnT{g}")
                for g in range(2)
            ]
            xa16 = [
                bpool.tile([ds, NTOK], F16, tag=f"xa{dc}", name=f"xa{dc}")
                for dc, (d0, ds) in enumerate(DC)
            ]
            # constant columns of v_s, written once before attention reads them
            for sc in range(3):
                nc.gpsimd.memset(v_s[sc][:, :, :, 16:32], 0.0)
                nc.gpsimd.memset(v_s[sc][:, :, :, 32:48], 1.0)
                nc.gpsimd.memset(v_s[sc][:, :, :, 48:64], 0.0)

            # ================= phase 1: qkv =================
            with (
                tc.tile_pool(name="xt", bufs=1) as xpool,
                tc.tile_pool(name="ps1", bufs=4, space="PSUM") as ps1,
            ):
                xt_s = [
                    xpool.tile([128, NTOK], F16, tag=f"xt{kc}", name=f"xt{kc}")
                    for kc in range(6)
                ]
                for kc, (k0, ks) in enumerate(K768):
                    nc.sync.dma_start(xt_s[kc][:, :], xt_d[k0 : k0 + ks, :])
                specs = ((qTs, 128, 0), (kTs, 128, 128), (vT, DHK, 256))
                for mi, (dst, mw, c0) in enumerate(specs):
                    for n0, ns in N5:
                        ps = ps1.tile([128, 480], F32, tag="qkvp")
                        for kc in range(6):
                            nc.tensor.matmul(
                                ps[:mw, :ns],
                                wqkv_s[kc][:, c0 : c0 + mw],
                                xt_s[kc][:, n0 : n0 + ns],
                                start=(kc == 0), stop=(kc == 5),
                            )
                        nc.scalar.activation(
                            dst[:, n0 : n0 + ns], ps[:mw, :ns], AF.Identity,
                            bias=qkvb_s[:mw, mi : mi + 1], scale=1.0,
                        )
                # move heads 2-3 down to base 0/32 (DMA shifts partitions)
                nc.sync.dma_start(qTb[:, :], qTs[64:128, :])
                nc.sync.dma_start(kTb[:, :], kTs[64:128, :])
                if debug:
                    dq = mpool.tile([128, NTOK], F32, tag="dbgq")
                    nc.vector.tensor_copy(dq[:, :], qTs[:, :])
                    nc.sync.dma_start(dbg_q[:, :], dq[:, :])
                    dk = mpool.tile([128, NTOK], F32, tag="dbgk")
                    nc.vector.tensor_copy(dk[:, :], kTs[:, :])
                    nc.sync.dma_start(dbg_k[:, :], dk[:, :])

            # ================= phase 2: attention =================
            if max_phase >= 2:
             with (
                tc.tile_pool(name="att", bufs=2) as apool,
                tc.tile_pool(name="ps2", bufs=2, space="PSUM") as ps2,
                tc.tile_pool(name="ps2z", bufs=2, space="PSUM") as ps2z,
            ):
                for b in range(BL):
                    base = b * T
                    # v = vT.T per s-chunk (all heads at once)
                    for sc, (s0, ss) in enumerate(TB):
                        vtr = ps2.tile([128, DHK], F16, tag="vtr")
                        nc.tensor.transpose(
                            vtr[:ss, :], vT[:, base + s0 : base + s0 + ss],
                            ident[0:DHK, 0:DHK],
                        )
                        nc.scalar.copy(
                            v_s[sc][0:ss, b, :, 0:16],
                            vtr[:ss, :].rearrange("p (h k) -> p h k", h=H),
                        )
                    # FT pass: scores [s, t] with the causal -30000 mask added
                    # into the diag block via an identity-stationary matmul;
                    # exp; then one matmul per (h, sc) computing
                    # [av_h | 0 | Z_h | 0] in a 64-row block; heads 2g / 2g+1
                    # at row bases 0 / 64 of azb[g]
                    azb = [
                        ps2z.tile([128, T], F32, tag=f"azb{g}", name=f"azb{g}")
                        for g in range(2)
                    ]
                    ft = {}
                    for h in range(H):
                        qt = qTs if h < 2 else qTb
                        kt = kTs if h < 2 else kTb
                        q0 = k0 = (h % 2) * 32
                        for sc, (s0, ss) in enumerate(TB):
                            tlen = T - s0
                            weiT = ps2.tile([128, T], F32, tag="weiT")
                            nc.tensor.matmul(
                                weiT[:ss, :tlen],
                                kt[k0 : k0 + HS, base + s0 : base + s0 + ss],
                                qt[q0 : q0 + HS, base + s0 : base + T],
                                start=True, stop=False,
                            )
                            nc.tensor.matmul(
                                weiT[:ss, :ss],
                                ident[0:ss, 0:ss],
                                mlt_s[0:ss, s0 : s0 + ss],
                                start=False, stop=True,
                                skip_group_check=True,
                            )
                            f = apool.tile([128, T], F16, tag=f"ft{h}{sc}")
                            ft[(h, sc)] = f
                            nc.scalar.activation(f[:ss, :tlen], weiT[:ss, :tlen], AF.Exp)
                        for sc, (s0, ss) in enumerate(TB):
                            nc.tensor.matmul(
                                azb[h // 2][64 * (h % 2) : 64 * (h % 2) + 64, s0:T],
                                v_s[sc][0:ss, b, h, :],
                                ft[(h, sc)][0:ss, 0 : T - s0],
                                start=(sc == 0), stop=(sc == 2),
                            )
                    for g in range(2):
                        rb = apool.tile([128, T], F32, tag=f"rb{g}")
                        nc.vector.reciprocal(rb[:, :], azb[g][:, :])
                        # attnT[g] = min(1/Z, RCLAMP) * av, compact 32-row
                        # blocks per head (av rows 0:16 valid, 16:32 zero)
                        nc.vector.scalar_tensor_tensor(
                            attnT[g][0:32, base : base + T],
                            rb[32:64, :], RCLAMP, azb[g][0:32, :],
                            op0=OP.min, op1=OP.mult,
                        )
                        nc.vector.scalar_tensor_tensor(
                            attnT[g][32:64, base : base + T],
                            rb[96:128, :], RCLAMP, azb[g][64:96, :],
                            op0=OP.min, op1=OP.mult,
                        )
                if debug:
                    for g in range(2):
                        da = apool.tile([64, NTOK], F32, tag=f"dbga{g}")
                        nc.vector.tensor_copy(da[:, :], attnT[g][:, :])
                        nc.sync.dma_start(dbg_at[64 * g : 64 * g + 64, :], da[:, :])

            # ================= phase 3: proj =================
            if max_phase >= 3:
             with tc.tile_pool(name="ps3", bufs=4, space="PSUM") as ps3:
                for dc, (d0, ds) in enumerate(DC):
                    for n0, ns in N5:
                        xap = ps3.tile([128, 480], F32, tag="xap")
                        for g in range(2):
                            nc.tensor.matmul(
                                xap[:ds, :ns], projw_s[g][:, d0 : d0 + ds],
                                attnT[g][:, n0 : n0 + ns],
                                start=(g == 0), stop=(g == 1),
                            )
                        nc.scalar.activation(
                            xa16[dc][:, n0 : n0 + ns], xap[:ds, :ns], AF.Identity,
                            bias=projb_s[dc][:, :], scale=1.0,
                        )
                if debug:
                    for dc, (d0, ds) in enumerate(DC):
                        dx = bpool.tile([ds, NTOK], F32, tag=f"dbgx{dc}")
                        nc.vector.tensor_copy(dx[:, :], xa16[dc][:, :])
                        nc.sync.dma_start(dbg_xa[d0 : d0 + ds, :], dx[:, :])

            _mid.close()
            _late = ExitStack()
            lpool = _late.enter_context(tc.tile_pool(name="late", bufs=1))
            gp = lpool.tile([128, len(TOK) * E], F32, tag="gp")
            G = lpool.tile([128, len(TOK) * D], F32, tag="G")
            S = lpool.tile([128, len(TOK) * D], F32, tag="S")

            # ================= phase 4: gating =================
            if max_phase >= 4:
             with (
                tc.tile_pool(name="gat", bufs=1) as gpool,
                tc.tile_pool(name="ps4", bufs=2, space="PSUM") as ps4,
                tc.tile_pool(name="ps4t", bufs=1, space="PSUM") as ps4t,
             ):
                NT_ = len(TOK)
                # logits transposed: [8, tok] with w_gate stationary
                logT = gpool.tile([8, NTOK], F16, tag="logT")
                for n0, ns in N5:
                    lg = ps4.tile([8, 480], F32, tag="lg")
                    for dc, (d0, ds) in enumerate(DC):
                        nc.tensor.matmul(
                            lg[:, :ns], wg_s[dc][:, :],
                            xa16[dc][:, n0 : n0 + ns],
                            start=(dc == 0), stop=(dc == 2),
                        )
                    nc.scalar.copy(logT[:, n0 : n0 + ns], lg[:, :ns])
                # transpose back to [tok, 8] in one psum bank
                ltp = ps4t.tile([128, NT_, E], F16, tag="ltp")
                for ti, (t0, ts_) in enumerate(TOK):
                    nc.tensor.transpose(
                        ltp[:ts_, ti, :], logT[:, t0 : t0 + ts_], ident[0:8, 0:8]
                    )
                L3 = gpool.tile([128, NT_, E], F32, tag="L3")
                nc.scalar.copy(L3[:, :, :], ltp[:, :, :])
                work = gpool.tile([128, NT_, E], F32, tag="work")
                nc.vector.tensor_copy(work[:, :, :], L3[:, :, :])
                m = gpool.tile([128, NT_, 4], F32, tag="m")
                eqm = gpool.tile([128, NT_, E], F32, tag="eqm")
                for r in range(TOPK):
                    nc.vector.tensor_reduce(
                        m[:, :, r : r + 1], work[:, :, :], axis=AX.X, op=OP.max,
                        opt_input=False, opt_output=False,
                    )
                    if r < TOPK - 1:
                        nc.vector.tensor_tensor(
                            eqm[:, :, :], work[:, :, :],
                            m[:, :, r : r + 1].broadcast_to((128, NT_, E)),
                            op=OP.is_equal,
                        )
                        nc.vector.scalar_tensor_tensor(
                            work[:, :, :], eqm[:, :, :], NEG, work[:, :, :],
                            op0=OP.mult, op1=OP.add,
                        )
                sel = gpool.tile([128, NT_, E], F32, tag="sel")
                nc.vector.tensor_tensor(
                    sel[:, :, :], L3[:, :, :],
                    m[:, :, 3:4].broadcast_to((128, NT_, E)), op=OP.is_ge,
                )
                el = gpool.tile([128, NT_, E], F32, tag="el")
                nc.scalar.activation(el[:, :, :], L3[:, :, :], AF.Exp)
                elm = gpool.tile([128, NT_, E], F32, tag="elm")
                nc.vector.tensor_tensor(elm[:, :, :], el[:, :, :], sel[:, :, :], op=OP.mult)
                zg = gpool.tile([128, NT_, 1], F32, tag="zg")
                nc.vector.tensor_reduce(
                    zg[:, :, :], elm[:, :, :], axis=AX.X, op=OP.add,
                    opt_input=False, opt_output=False,
                )
                rzg = gpool.tile([128, NT_, 1], F32, tag="rzg")
                nc.vector.reciprocal(rzg[:, :, :], zg[:, :, :])
                nc.vector.tensor_tensor(
                    gp[:, :].rearrange("p (t e) -> p t e", e=E), elm[:, :, :],
                    rzg[:, :, :].broadcast_to((128, NT_, E)),
                    op=OP.mult,
                )
            if debug and max_phase >= 4:
                nc.sync.dma_start(dbg_gp[:, :], gp[:, :])

            # ================= phase 5: experts =================
            if max_phase >= 5:
             with (
                tc.tile_pool(name="outp", bufs=4) as opool,
                tc.tile_pool(name="wexp", bufs=1) as wpool,
                tc.tile_pool(name="ht", bufs=2) as hpool,
                tc.tile_pool(name="ps5", bufs=1, space="PSUM") as ps5h,
                tc.tile_pool(name="ps5b", bufs=3, space="PSUM") as ps5e,
            ):
                for e in range(E // 2, E):
                    w1t = []
                    for dc, (d0, ds) in enumerate(DC):
                        t = wpool.tile([ds, DH], F16, tag=f"w1_{e}_{dc}", name=f"w1_{e}_{dc}")
                        nc.gpsimd.dma_start(t[:, :], w1_d[e, d0 : d0 + ds, :])
                        w1t.append(t)
                    w1_all.append(w1t)
                    w2t = []
                    for mc, (h0, hs) in enumerate(MCH):
                        t = wpool.tile([hs, D], F16, tag=f"w2_{e}_{mc}", name=f"w2_{e}_{mc}")
                        nc.gpsimd.dma_start(t[:, :], w2_d[e, h0 : h0 + hs, :])
                        w2t.append(t)
                    w2_all.append(w2t)
                    if b2_nonzero:
                        t = wpool.tile([1, D], F16, tag=f"b2row{e}", name=f"b2row{e}")
                        nc.gpsimd.dma_start(t[:, :], b2_d[e, :, :])
                        b2_all.append(t)
                ones = None
                if b2_nonzero:
                    ones = cpool.tile([1, NTOK], F16, tag="ones")
                    nc.gpsimd.memset(ones[:, :], 1.0)
                for e in range(E):
                    w1t = w1_all[e]
                    w2t = w2_all[e]
                    b2row = b2_all[e] if b2_nonzero else None
                    ht = []
                    for mc, (h0, hs) in enumerate(MCH):
                        t = hpool.tile([hs, NTOK], F16, tag=f"ht{mc}")
                        ht.append(t)
                        hps = [
                            ps5h.tile([128, 480], F32, tag=f"hp{n}", name=f"hp{n}")
                            for n in range(5)
                        ]
                        for dc in range(3):
                            for n, (n0, ns) in enumerate(N5):
                                nc.tensor.matmul(
                                    hps[n][:hs, :ns], w1t[dc][:, h0 : h0 + hs],
                                    xa16[dc][:, n0 : n0 + ns],
                                    start=(dc == 0), stop=(dc == 2),
                                )
                        for n, (n0, ns) in enumerate(N5):
                            nc.scalar.activation(
                                t[:, n0 : n0 + ns], hps[n][:hs, :ns], AF.Relu,
                                bias=b1_s[:hs, e * 5 + mc : e * 5 + mc + 1], scale=1.0,
                            )
                    for ti, (t0, ts_) in enumerate(TOK):
                        eo = ps5e.tile([128, D], F32, tag="eo")
                        for mc in range(5):
                            nc.tensor.matmul(
                                eo[:ts_, :], ht[mc][:, t0 : t0 + ts_], w2t[mc][:, :],
                                start=(mc == 0), stop=(mc == 4 and not b2_nonzero),
                            )
                        if b2_nonzero:
                            nc.tensor.matmul(
                                eo[:ts_, :], ones[:, t0 : t0 + ts_], b2row[:, :],
                                start=False, stop=True,
                            )
                        gsc = gp[:ts_, ti * E + e : ti * E + e + 1]
                        gsl = G[:ts_, ti * D : (ti + 1) * D]
                        ssl = S[:ts_, ti * D : (ti + 1) * D]
                        if e == 0:
                            nc.vector.tensor_scalar_mul(gsl, eo[:ts_, :], gsc)
                            nc.scalar.copy(ssl, eo[:ts_, :])
                        else:
                            nc.vector.scalar_tensor_tensor(
                                gsl, eo[:ts_, :], gsc, gsl, op0=OP.mult, op1=OP.add
                            )
                            nc.vector.tensor_tensor(ssl, eo[:ts_, :], ssl, op=OP.add)
                        if e == E - 1:
                            o = opool.tile([128, D], F32, tag="o")
                            nc.vector.tensor_tensor(o[:ts_, :], gsl, ssl, op=OP.mult)
                            nc.sync.dma_start(out_d[t0 : t0 + ts_, :], o[:ts_, :])

            _late.close()

    _split_waits(nc)
    return nc


_CACHE = {}
LAST_RESULT = None


def _get_module(b2_nonzero: bool, debug: bool = False, max_phase: int = 9):
    key = (b2_nonzero, debug, max_phase)
    if key not in _CACHE:
        _CACHE[key] = _build_module(b2_nonzero, debug=debug, max_phase=max_phase)
    return _CACHE[key]


def _prep_inputs(x, wh_bias, wh_W, Wq, Wk, Wv, proj_W, proj_b,
                 exp_W1, exp_b1, exp_W2, exp_b2, w_gate):
    # fold whiten into qkv; fold attention scale into q
    scale = float(D) ** -0.5
    Wqf = (Wq.reshape(DHK, D) @ wh_W) * scale          # [64, 768]
    Wkf = Wk.reshape(DHK, D) @ wh_W
    Wvf = Wv.reshape(DHK, D) @ wh_W

    def stripe(w):                                     # [64, 768] -> [128, 768]
        out = np.zeros((128, DIN), np.float32)
        for h in range(H):
            out[h * 32 : h * 32 + HS] = w[h * HS : (h + 1) * HS]
        return out

    def stripe_b(v):                                   # [64] -> [128]
        out = np.zeros(128, np.float32)
        for h in range(H):
            out[h * 32 : h * 32 + HS] = v[h * HS : (h + 1) * HS]
        return out

    wqkv = np.concatenate([stripe(Wqf), stripe(Wkf), Wvf], 0)   # [320, 768]
    bq = -(Wqf @ wh_bias)
    bk = -(Wkf @ wh_bias)
    bv = -(Wvf @ wh_bias)
    qkvb = np.stack([stripe_b(bq), stripe_b(bk), np.pad(bv, (0, 64))], 1)  # [128, 3]

    # proj stationary per head-pair group: rows 0:16 = head 2g, 32:48 =
    # head 2g+1, rest zero (matches attnT zero rows)
    projwS = np.zeros((2, 64, D), np.float32)
    for h in range(H):
        projwS[h // 2, 32 * (h % 2) : 32 * (h % 2) + HS] = (
            proj_W[:, h * HS : (h + 1) * HS].T
        )

    # causal mask diag blocks: -30000 strictly below the diagonal (s > t)
    mlt = np.zeros((128, T), np.float32)
    for s0, ss in TB:
        mlt[:ss, s0 : s0 + ss] = np.tril(
            np.full((ss, ss), -30000.0, np.float32), -1
        )

    # b1 bias slices [128, E*5]: column e*5+mc holds exp_b1[e, h0:h0+hs]
    b1s = np.zeros((128, E * 5), np.float32)
    for e in range(E):
        for mc, (h0, hs) in enumerate(MCH):
            b1s[:hs, e * 5 + mc] = exp_b1[e, h0 : h0 + hs]

    f16 = np.float16
    common = {
        "wqkv": np.ascontiguousarray(wqkv.T).astype(f16),
        "qkvb": np.ascontiguousarray(qkvb).astype(np.float32),
        "projw": np.ascontiguousarray(projwS).astype(f16),
        "projb": np.ascontiguousarray(proj_b[:, None]).astype(np.float32),
        "wg": np.ascontiguousarray(w_gate).astype(f16),
        "mlt": mlt.astype(f16),
        "idm": np.eye(128, dtype=f16),
        "w1t": np.ascontiguousarray(exp_W1.transpose(0, 2, 1)).astype(f16),
        "w2t": np.ascontiguousarray(exp_W2.transpose(0, 2, 1)).astype(f16),
        "b1s": b1s.astype(np.float32),
        "b2s": np.ascontiguousarray(exp_b2[:, None, :]).astype(f16),
    }
    in_maps = []
    for c in range(NCORES):
        xc = x[c * BL : (c + 1) * BL]                  # [8, 300, 768]
        xt = np.ascontiguousarray(
            xc.transpose(2, 0, 1).reshape(DIN, NTOK)
        ).astype(f16)
        in_maps.append({**common, "xt": xt})
    return in_maps


def kernel(x, wh_bias, wh_W, Wq, Wk, Wv, proj_W, proj_b,
           exp_W1, exp_b1, exp_W2, exp_b2, w_gate,
           debug=False, max_phase=9):
    global LAST_RESULT
    x = np.asarray(x, np.float32)
    wh_bias = np.asarray(wh_bias, np.float32)
    wh_W = np.asarray(wh_W, np.float32)
    Wq, Wk, Wv = (np.asarray(w, np.float32) for w in (Wq, Wk, Wv))
    proj_W = np.asarray(proj_W, np.float32)
    proj_b = np.asarray(proj_b, np.float32)
    exp_W1 = np.asarray(exp_W1, np.float32)
    exp_b1 = np.asarray(exp_b1, np.float32)
    exp_W2 = np.asarray(exp_W2, np.float32)
    exp_b2 = np.asarray(exp_b2, np.float32)
    w_gate = np.asarray(w_gate, np.float32)

    b2_nonzero = bool(np.any(exp_b2))
    in_maps = _prep_inputs(x, wh_bias, wh_W, Wq, Wk, Wv, proj_W, proj_b,
                           exp_W1, exp_b1, exp_W2, exp_b2, w_gate)

    nc = _get_module(b2_nonzero, debug=debug, max_phase=max_phase)
    for alloc in nc.m.functions[0].allocations:
        if isinstance(alloc, mybir.MemoryLocationSet) and alloc.kind == "ExternalInput":
            nm = alloc.memorylocations[0].name
            if nm not in in_maps[0]:
                continue  # partition_id etc., supplied by the runner
            got = in_maps[0][nm]
            assert tuple(got.shape) == tuple(alloc.tensor_shape), (
                nm, got.shape, alloc.tensor_shape)
            assert got.dtype == mybir.dt.np(alloc.dtype), (nm, got.dtype)
    res = run_bass_kernel_spmd(nc, in_maps, core_ids=list(range(NCORES)))
    LAST_RESULT = res
    out = np.stack([r["out"] for r in res.results])    # [8, 2400, 300]
    return out.reshape(B, T, D)


# revision 30
# speedup vs baseline: 1.7355x; 1.5145x over previous
"""Trainium2 Bass kernel for nn_MoEAdaptorLayer (whiten -> causal MHA -> proj
-> noisy-top-k gating (eval) -> 8 dense experts -> gated mixture * expert sum).

Sharding: data-parallel over batch. 64 batches -> 8 per core, params replicated.

v2 redesign vs baseline:
- fp16 everywhere on the matmul path (fp32 PSUM accumulation); halves DMA and
  removes the fp32r small-free-dim PE penalties.
- qkv: x shipped as one [768, 2400] fp16 tensor per core (6 big DMAs), matmuls
  in [128, 480] chunks.
- attention: single-orientation flash-style pass. Scores computed [s, t] only;
  exp on scalar; causal mask by multiplying the diagonal blocks with a
  triangular constant (split vector/gpsimd). AV and the softmax denominator
  come from ONE matmul per (head, s-chunk): stationary [v_h | ones] (32 cols)
  so the psum holds [av_h (16 rows) | Z_h replicated (16 rows)] per 32-stripe.
  One reciprocal + one fused min-mult STT per batch produce attnT directly in
  head-striped [128, tok] layout (no Z-pass, no output transposes).
- proj: stationary is zero-padded to the 32-stripe layout; 15 fp16 matmuls.
- gating: logits computed transposed ([8, tok], weight-stationary, 15 matmuls
  instead of 57 LDW-bound ones), PE-transposed back in 19 tiny transposes.
- experts: h chunked {128,128,128,128,88} (M=128-aligned LDWEIGHTS is ~2x
  faster than M=120), fp16 weights/activations.
"""

from contextlib import ExitStack

import numpy as np

import concourse.bass as bass
import concourse.tile as tile
import concourse.mybir as mybir
from concourse.bass_utils import run_bass_kernel_spmd

F16 = mybir.dt.float16
F32 = mybir.dt.float32
AX = mybir.AxisListType
OP = mybir.AluOpType
AF = mybir.ActivationFunctionType

B, T, DIN, D, E, H, HS = 64, 300, 768, 300, 8, 4, 16
NCORES = 8
BL = B // NCORES          # 8 batches per core
NTOK = BL * T             # 2400 tokens per core
DHK = H * HS              # 64
DH = 2 * D                # 600
TOPK = E // 2

K768 = [(i * 128, 128) for i in range(6)]
DC = [(0, 128), (128, 128), (256, 44)]          # 300 = 128+128+44
MCH = [(0, 128), (128, 128), (256, 128), (384, 128), (512, 88)]  # 600
TB = [(0, 128), (128, 128), (256, 44)]          # 300 tokens per batch
TOK = [(i * 128, 128) for i in range(18)] + [(2304, 96)]   # 2400 tokens
N5 = [(i * 480, 480) for i in range(5)]         # 2400 free-dim chunks
NEG = -1.0e30
RCLAMP = 16.0   # cap on 1/Z; keeps garbage stripe rows finite in fp16

_MAX_DRAIN_WAITS = 1
_WAIT_LIMIT = 1


def _split_waits(nc):
    """Walrus in this build caps sync waits per instruction; hoist excess
    waits onto same-engine NOPs inserted just before the instruction."""
    n = 0
    for f in nc.m.functions:
        for blk in f.blocks:
            insts = blk.instructions
            out = []
            changed = False
            for inst in insts:
                si = inst.sync_info
                waits = list(si.on_wait or []) if si is not None else []
                if len(waits) > _WAIT_LIMIT:
                    head, tail = waits[:-_WAIT_LIMIT], waits[-_WAIT_LIMIT:]
                    for i in range(0, len(head), _WAIT_LIMIT):
                        n += 1
                        nop = mybir.InstNoOp(name=f"waitnop{n}", ins=[], outs=[])
                        nop.engine = inst.engine
                        nop.sync_info = mybir.SyncInfo(
                            on_wait=head[i : i + _WAIT_LIMIT], on_update=[]
                        )
                        out.append(nop)
                    si.on_wait = tail
                    inst.sync_info = si
                    changed = True
                out.append(inst)
            if changed:
                blk.instructions = out


def _install_drain_patch():
    """This walrus build rejects CTRL instructions with more than a few sync
    waits; Tile's tail drain waits on every engine/DMA semaphore at once.
    Split the waits across a chain of single-wait drains."""
    if getattr(tile.TileContext, "_drain_patched", False):
        return

    def _patched(self, tick_clock, wait_clock):
        nc = self.nc
        drain_inst = nc.sync.drain()
        wait_clock.add_sem_waits(
            drain_inst.ins, tile.ScopedClock({None: tick_clock.global_clock})
        )
        ri = drain_inst.ins
        si = ri.sync_info
        waits = list(si.on_wait or []) if si is not None else []
        if len(waits) > _MAX_DRAIN_WAITS:
            si.on_wait = waits[:_MAX_DRAIN_WAITS]
            ri.sync_info = si
            for i in range(_MAX_DRAIN_WAITS, len(waits), _MAX_DRAIN_WAITS):
                d2 = nc.sync.drain()
                d2.ins.sync_info = mybir.SyncInfo(
                    on_wait=waits[i : i + _MAX_DRAIN_WAITS], on_update=[]
                )
        nc.all_engine_barrier()
        assert self.sems is not None
        popped = nc._tile_sem_poison_stack.pop()
        assert popped is self._sem_poison
        nc.clear_and_free_semaphores(list(self.sems.allocated().values()))
        nc.all_engine_barrier()

    tile.TileContext._drain_and_barrier = _patched
    tile.TileContext._drain_patched = True


def _build_module(b2_nonzero: bool, debug: bool = False, max_phase: int = 9):
    _install_drain_patch()
    nc = bass.Bass("TRN2", target_bir_lowering=False, debug=False)

    # ---- DRAM I/O ----
    xt_d = nc.dram_tensor("xt", [DIN, NTOK], F16, kind="ExternalInput")
    wqkv_d = nc.dram_tensor("wqkv", [DIN, 320], F16, kind="ExternalInput")
    qkvb_d = nc.dram_tensor("qkvb", [128, 3], F32, kind="ExternalInput")
    wgf_d = nc.dram_tensor("wgf", [128, E], F16, kind="ExternalInput")
    cb_d = nc.dram_tensor("cb", [E, 1], F32, kind="ExternalInput")
    w1_d = nc.dram_tensor("w1t", [E, 128, DH], F16, kind="ExternalInput")
    w2_d = nc.dram_tensor("w2t", [E, DH, D], F16, kind="ExternalInput")
    b1_d = nc.dram_tensor("b1s", [128, E * 5], F32, kind="ExternalInput")
    b2_d = nc.dram_tensor("b2s", [E, 1, D], F16, kind="ExternalInput")
    idm_d = nc.dram_tensor("idm", [128, 128], F16, kind="ExternalInput")
    out_d = nc.dram_tensor("out", [NTOK, D], F32, kind="ExternalOutput")
    if debug:
        dbg_q = nc.dram_tensor("dbg_q", [128, NTOK], F32, kind="ExternalOutput")
        dbg_k = nc.dram_tensor("dbg_k", [128, NTOK], F32, kind="ExternalOutput")
        dbg_at = nc.dram_tensor("dbg_at", [128, NTOK], F32, kind="ExternalOutput")
        dbg_xa = nc.dram_tensor("dbg_xa", [D, NTOK], F32, kind="ExternalOutput")
        dbg_gp = nc.dram_tensor("dbg_gp", [128, len(TOK) * E], F32, kind="ExternalOutput")

    with tile.TileContext(nc) as tc:
        with (
            tc.tile_pool(name="const", bufs=1) as cpool,
            tc.tile_pool(name="big", bufs=1) as bpool,
        ):
            # ---- persistent constants ----
            ident = cpool.tile([128, 128], F16)
            nc.sync.dma_start(ident[:, :], idm_d[:, :])
            wqkv_s = []
            for kc, (k0, ks) in enumerate(K768):
                t = cpool.tile([128, 320], F16, tag=f"wqkv{kc}")
                nc.sync.dma_start(t[:, :], wqkv_d[k0 : k0 + ks, :])
                wqkv_s.append(t)
            qkvb_s = cpool.tile([128, 3], F32, tag="qkvbs")
            nc.sync.dma_start(qkvb_s[:, :], qkvb_d[:, :])
            wgf_s = cpool.tile([128, E], F16, tag="wgf")
            nc.sync.dma_start(wgf_s[:, :], wgf_d[:, :])
            cb_s = cpool.tile([E, 1], F32, tag="cb")
            nc.sync.dma_start(cb_s[:, :], cb_d[:, :])
            b1_s = cpool.tile([128, E * 5], F32, tag="b1s")
            nc.sync.dma_start(b1_s[:, :], b1_d[:, :])

            # prefetch experts 0-3 weights up front on the gpsimd DMA queue
            w1_all, w2_all, b2_all = [], [], []
            for e in range(E // 2):
                t = bpool.tile([128, DH], F16, tag=f"w1_{e}", name=f"w1_{e}")
                nc.gpsimd.dma_start(t[:, :], w1_d[e, :, :])
                w1_all.append(t)
                w2t = []
                for mc, (h0, hs) in enumerate(MCH):
                    t = bpool.tile([hs, D], F16, tag=f"w2_{e}_{mc}", name=f"w2_{e}_{mc}")
                    nc.gpsimd.dma_start(t[:, :], w2_d[e, h0 : h0 + hs, :])
                    w2t.append(t)
                w2_all.append(w2t)
                if b2_nonzero:
                    t = bpool.tile([1, D], F16, tag=f"b2row{e}", name=f"b2row{e}")
                    nc.gpsimd.dma_start(t[:, :], b2_d[e, :, :])
                    b2_all.append(t)

            # ---- activations ----
            _mid = ExitStack()
            mpool = _mid.enter_context(tc.tile_pool(name="mid", bufs=1))
            # heads striped at 32-partition bases (rows 32h:32h+16 per head)
            qTs = mpool.tile([128, NTOK], F16, tag="qTs")
            kTs = mpool.tile([128, NTOK], F16, tag="kTs")
            qTb = mpool.tile([DHK, NTOK], F16, tag="qTb")
            kTb = mpool.tile([DHK, NTOK], F16, tag="kTb")
            vT = mpool.tile([DHK, NTOK], F16, tag="vT")
            # v_s[sc]: [ss, b, h, 64] = [v_h(16) | 0(16) | ones(16) | 0(16)];
            # the av/Z layout this produces keeps every vector-op partition
            # base 32-aligned
            v_s = [
                mpool.tile([128, BL, H, 64], F16, tag=f"v{sc}", name=f"v{sc}")
                for sc in range(3)
            ]
            # attnT [128, NTOK]: rows 32h:32h+16 = head h, other rows are
            # the constant 1.0 (Z/Z); folded weights are zero there
            attnT = bpool.tile([128, NTOK], F16, tag="attnT")
            # v_s constant columns [v16 | ones48]: one matmul per (h, sc)
            # produces [av (16 rows) | Z replicated (48 rows)]
            for sc in range(3):
                nc.gpsimd.memset(v_s[sc][:, :, :, 16:64], 1.0)

            # ================= phase 1: qkv =================
            with (
                tc.tile_pool(name="xt", bufs=1) as xpool,
                tc.tile_pool(name="ps1", bufs=4, space="PSUM") as ps1,
            ):
                xt_s = [
                    xpool.tile([128, NTOK], F16, tag=f"xt{kc}", name=f"xt{kc}")
                    for kc in range(6)
                ]
                for kc, (k0, ks) in enumerate(K768):
                    nc.sync.dma_start(xt_s[kc][:, :], xt_d[k0 : k0 + ks, :])
                specs = ((qTs, 128, 0), (kTs, 128, 128), (vT, DHK, 256))
                for mi, (dst, mw, c0) in enumerate(specs):
                    for n0, ns in N5:
                        ps = ps1.tile([128, 480], F32, tag="qkvp")
                        for kc in range(6):
                            nc.tensor.matmul(
                                ps[:mw, :ns],
                                wqkv_s[kc][:, c0 : c0 + mw],
                                xt_s[kc][:, n0 : n0 + ns],
                                start=(kc == 0), stop=(kc == 5),
                            )
                        nc.scalar.activation(
                            dst[:, n0 : n0 + ns], ps[:mw, :ns], AF.Identity,
                            bias=qkvb_s[:mw, mi : mi + 1], scale=1.0,
                        )
                # move heads 2-3 down to base 0/32 (DMA shifts partitions)
                nc.sync.dma_start(qTb[:, :], qTs[64:128, :])
                nc.sync.dma_start(kTb[:, :], kTs[64:128, :])
                if debug:
                    dq = mpool.tile([128, NTOK], F32, tag="dbgq")
                    nc.vector.tensor_copy(dq[:, :], qTs[:, :])
                    nc.sync.dma_start(dbg_q[:, :], dq[:, :])
                    dk = mpool.tile([128, NTOK], F32, tag="dbgk")
                    nc.vector.tensor_copy(dk[:, :], kTs[:, :])
                    nc.sync.dma_start(dbg_k[:, :], dk[:, :])

            # ================= phase 2: attention =================
            if max_phase >= 2:
             with (
                tc.tile_pool(name="att", bufs=2) as apool,
                tc.tile_pool(name="ps2", bufs=2, space="PSUM") as ps2,
                tc.tile_pool(name="ps2z", bufs=2, space="PSUM") as ps2z,
            ):
                for b in range(BL):
                    base = b * T
                    # v = vT.T per s-chunk (all heads at once)
                    for sc, (s0, ss) in enumerate(TB):
                        vtr = ps2.tile([128, DHK], F16, tag="vtr")
                        nc.tensor.transpose(
                            vtr[:ss, :], vT[:, base + s0 : base + s0 + ss],
                            ident[0:DHK, 0:DHK],
                        )
                        nc.scalar.copy(
                            v_s[sc][0:ss, b, :, 0:16],
                            vtr[:ss, :].rearrange("p (h k) -> p h k", h=H),
                        )
                    # FT pass: scores [s, t] with the causal -30000 mask added
                    # into the diag block via an identity-stationary matmul;
                    # exp; then one matmul per (h, sc) computing
                    # [av_h | 0 | Z_h | 0] in a 64-row block; heads 2g / 2g+1
                    # at row bases 0 / 64 of azb[g]
                    azb = [
                        ps2z.tile([128, T], F32, tag=f"azb{g}", name=f"azb{g}")
                        for g in range(2)
                    ]
                    ft = {}
                    for h in range(H):
                        qt = qTs if h < 2 else qTb
                        kt = kTs if h < 2 else kTb
                        q0 = k0 = (h % 2) * 32
                        for sc, (s0, ss) in enumerate(TB):
                            tlen = T - s0
                            weiT = ps2.tile([128, T], F32, tag="weiT")
                            nc.tensor.matmul(
                                weiT[:ss, :tlen],
                                kt[k0 : k0 + HS, base + s0 : base + s0 + ss],
                                qt[q0 : q0 + HS, base + s0 : base + T],
                                start=True, stop=True,
                            )
                            f = apool.tile([128, T], F16, tag=f"ft{h}{sc}")
                            ft[(h, sc)] = f
                            nc.scalar.activation(f[:ss, :tlen], weiT[:ss, :tlen], AF.Exp)
                            nc.gpsimd.affine_select(
                                out=f[:ss, :ss], in_=f[:ss, :ss],
                                compare_op=OP.is_ge, fill=0.0,
                                base=0, channel_multiplier=-1, pattern=[[1, ss]],
                            )
                        for sc, (s0, ss) in enumerate(TB):
                            nc.tensor.matmul(
                                azb[h // 2][64 * (h % 2) : 64 * (h % 2) + 64, s0:T],
                                v_s[sc][0:ss, b, h, :],
                                ft[(h, sc)][0:ss, 0 : T - s0],
                                start=(sc == 0), stop=(sc == 2),
                            )
                    # 1/azb on the scalar table engine (cheap); only the
                    # Z-recip rows are ever read back, so the garbage
                    # av-recip rows (possibly inf) are harmless
                    rz = [
                        apool.tile([128, T], F32, tag=f"rz{g}", name=f"rz{g}")
                        for g in range(2)
                    ]
                    for g in range(2):
                        nc.vector.reciprocal(rz[g][:, :], azb[g][:, :])
                    for h in range(H):
                        g, o = h // 2, 64 * (h % 2)
                        nc.vector.scalar_tensor_tensor(
                            attnT[32 * h : 32 * h + 32, base : base + T],
                            rz[g][o + 32 : o + 64, :], RCLAMP,
                            azb[g][o : o + 32, :],
                            op0=OP.min, op1=OP.mult,
                        )
                if debug:
                    da = apool.tile([128, NTOK], F32, tag="dbga")
                    nc.vector.tensor_copy(da[:, :], attnT[:, :])
                    nc.sync.dma_start(dbg_at[:, :], da[:, :])

            _mid.close()
            _late = ExitStack()
            lpool = _late.enter_context(tc.tile_pool(name="late", bufs=1))
            gp = lpool.tile([128, len(TOK) * E], F32, tag="gp")
            G = lpool.tile([128, len(TOK) * D], F32, tag="G")
            S = lpool.tile([128, len(TOK) * D], F32, tag="S")

            # ================= phase 4: gating =================
            if max_phase >= 4:
             with (
                tc.tile_pool(name="gat", bufs=1) as gpool,
                tc.tile_pool(name="ps4", bufs=2, space="PSUM") as ps4,
                tc.tile_pool(name="ps4t", bufs=1, space="PSUM") as ps4t,
             ):
                NT_ = len(TOK)
                # logits transposed: [8, tok], folded proj+gate stationary
                logT = gpool.tile([8, NTOK], F16, tag="logT")
                for n0, ns in N5:
                    lg = ps4.tile([8, 480], F32, tag="lg")
                    nc.tensor.matmul(
                        lg[:, :ns], wgf_s[:, :], attnT[:, n0 : n0 + ns],
                        start=True, stop=True,
                    )
                    nc.scalar.activation(
                        logT[:, n0 : n0 + ns], lg[:, :ns], AF.Identity,
                        bias=cb_s[:, :], scale=1.0,
                    )
                # transpose back to [tok, 8] in one psum bank
                ltp = ps4t.tile([128, NT_, E], F16, tag="ltp")
                for ti, (t0, ts_) in enumerate(TOK):
                    nc.tensor.transpose(
                        ltp[:ts_, ti, :], logT[:, t0 : t0 + ts_], ident[0:8, 0:8]
                    )
                L3 = gpool.tile([128, NT_, E], F32, tag="L3")
                nc.scalar.copy(L3[:, :, :], ltp[:, :, :])
                work = gpool.tile([128, NT_, E], F32, tag="work")
                nc.vector.tensor_copy(work[:, :, :], L3[:, :, :])
                m = gpool.tile([128, NT_, 4], F32, tag="m")
                eqm = gpool.tile([128, NT_, E], F32, tag="eqm")
                for r in range(TOPK):
                    nc.vector.tensor_reduce(
                        m[:, :, r : r + 1], work[:, :, :], axis=AX.X, op=OP.max,
                        opt_input=False, opt_output=False,
                    )
                    if r < TOPK - 1:
                        nc.vector.tensor_tensor(
                            eqm[:, :, :], work[:, :, :],
                            m[:, :, r : r + 1].broadcast_to((128, NT_, E)),
                            op=OP.is_equal,
                        )
                        nc.vector.scalar_tensor_tensor(
                            work[:, :, :], eqm[:, :, :], NEG, work[:, :, :],
                            op0=OP.mult, op1=OP.add,
                        )
                sel = gpool.tile([128, NT_, E], F32, tag="sel")
                nc.vector.tensor_tensor(
                    sel[:, :, :], L3[:, :, :],
                    m[:, :, 3:4].broadcast_to((128, NT_, E)), op=OP.is_ge,
                )
                el = gpool.tile([128, NT_, E], F32, tag="el")
                nc.scalar.activation(el[:, :, :], L3[:, :, :], AF.Exp)
                elm = gpool.tile([128, NT_, E], F32, tag="elm")
                nc.vector.tensor_tensor(elm[:, :, :], el[:, :, :], sel[:, :, :], op=OP.mult)
                zg = gpool.tile([128, NT_, 1], F32, tag="zg")
                nc.vector.tensor_reduce(
                    zg[:, :, :], elm[:, :, :], axis=AX.X, op=OP.add,
                    opt_input=False, opt_output=False,
                )
                rzg = gpool.tile([128, NT_, 1], F32, tag="rzg")
                nc.vector.reciprocal(rzg[:, :, :], zg[:, :, :])
                nc.vector.tensor_tensor(
                    gp[:, :].rearrange("p (t e) -> p t e", e=E), elm[:, :, :],
                    rzg[:, :, :].broadcast_to((128, NT_, E)),
                    op=OP.mult,
                )
            if debug and max_phase >= 4:
                nc.sync.dma_start(dbg_gp[:, :], gp[:, :])

            # ================= phase 5: experts =================
            if max_phase >= 5:
             with (
                tc.tile_pool(name="outp", bufs=4) as opool,
                tc.tile_pool(name="wexp", bufs=1) as wpool,
                tc.tile_pool(name="ht", bufs=2) as hpool,
                tc.tile_pool(name="ps5", bufs=3, space="PSUM") as ps5h,
                tc.tile_pool(name="ps5b", bufs=5, space="PSUM") as ps5e,
            ):
                for e in range(E // 2, E):
                    t = wpool.tile([128, DH], F16, tag=f"w1_{e}", name=f"w1_{e}")
                    nc.gpsimd.dma_start(t[:, :], w1_d[e, :, :])
                    w1_all.append(t)
                    w2t = []
                    for mc, (h0, hs) in enumerate(MCH):
                        t = wpool.tile([hs, D], F16, tag=f"w2_{e}_{mc}", name=f"w2_{e}_{mc}")
                        nc.gpsimd.dma_start(t[:, :], w2_d[e, h0 : h0 + hs, :])
                        w2t.append(t)
                    w2_all.append(w2t)
                    if b2_nonzero:
                        t = wpool.tile([1, D], F16, tag=f"b2row{e}", name=f"b2row{e}")
                        nc.gpsimd.dma_start(t[:, :], b2_d[e, :, :])
                        b2_all.append(t)
                ones = None
                if b2_nonzero:
                    ones = cpool.tile([1, NTOK], F16, tag="ones")
                    nc.gpsimd.memset(ones[:, :], 1.0)
                for e in range(E):
                    w1t = w1_all[e]
                    w2t = w2_all[e]
                    b2row = b2_all[e] if b2_nonzero else None
                    ht = []
                    for mc, (h0, hs) in enumerate(MCH):
                        t = hpool.tile([hs, NTOK], F16, tag=f"ht{mc}")
                        ht.append(t)
                        for n, (n0, ns) in enumerate(N5):
                            hp = ps5h.tile([128, 480], F32, tag="hp")
                            nc.tensor.matmul(
                                hp[:hs, :ns], w1t[:, h0 : h0 + hs],
                                attnT[:, n0 : n0 + ns],
                                start=True, stop=True,
                            )
                            nc.scalar.activation(
                                t[:, n0 : n0 + ns], hp[:hs, :ns], AF.Relu,
                                bias=b1_s[:hs, e * 5 + mc : e * 5 + mc + 1], scale=1.0,
                            )
                    for ti, (t0, ts_) in enumerate(TOK):
                        eo = ps5e.tile([128, D], F32, tag="eo")
                        for mc in range(5):
                            nc.tensor.matmul(
                                eo[:ts_, :], ht[mc][:, t0 : t0 + ts_], w2t[mc][:, :],
                                start=(mc == 0), stop=(mc == 4 and not b2_nonzero),
                            )
                        if b2_nonzero:
                            nc.tensor.matmul(
                                eo[:ts_, :], ones[:, t0 : t0 + ts_], b2row[:, :],
                                start=False, stop=True,
                            )
                        gsc = gp[:ts_, ti * E + e : ti * E + e + 1]
                        gsl = G[:ts_, ti * D : (ti + 1) * D]
                        ssl = S[:ts_, ti * D : (ti + 1) * D]
                        if e == 0:
                            nc.vector.tensor_scalar_mul(gsl, eo[:ts_, :], gsc)
                            nc.scalar.copy(ssl, eo[:ts_, :])
                        else:
                            nc.vector.scalar_tensor_tensor(
                                gsl, eo[:ts_, :], gsc, gsl, op0=OP.mult, op1=OP.add
                            )
                            nc.vector.tensor_tensor(ssl, eo[:ts_, :], ssl, op=OP.add)
                        if e == E - 1:
                            o = opool.tile([128, D], F32, tag="o")
                            nc.vector.tensor_tensor(o[:ts_, :], gsl, ssl, op=OP.mult)
                            nc.sync.dma_start(out_d[t0 : t0 + ts_, :], o[:ts_, :])

            _late.close()

    _split_waits(nc)
    return nc


_CACHE = {}
LAST_RESULT = None


def _get_module(b2_nonzero: bool, debug: bool = False, max_phase: int = 9):
    key = (b2_nonzero, debug, max_phase)
    if key not in _CACHE:
        _CACHE[key] = _build_module(b2_nonzero, debug=debug, max_phase=max_phase)
    return _CACHE[key]


def _prep_inputs(x, wh_bias, wh_W, Wq, Wk, Wv, proj_W, proj_b,
                 exp_W1, exp_b1, exp_W2, exp_b2, w_gate):
    # fold whiten into qkv; fold attention scale into q
    scale = float(D) ** -0.5
    Wqf = (Wq.reshape(DHK, D) @ wh_W) * scale          # [64, 768]
    Wkf = Wk.reshape(DHK, D) @ wh_W
    Wvf = Wv.reshape(DHK, D) @ wh_W

    def stripe(w):                                     # [64, 768] -> [128, 768]
        out = np.zeros((128, DIN), np.float32)
        for h in range(H):
            out[h * 32 : h * 32 + HS] = w[h * HS : (h + 1) * HS]
        return out

    def stripe_b(v):                                   # [64] -> [128]
        out = np.zeros(128, np.float32)
        for h in range(H):
            out[h * 32 : h * 32 + HS] = v[h * HS : (h + 1) * HS]
        return out

    wqkv = np.concatenate([stripe(Wqf), stripe(Wkf), Wvf], 0)   # [320, 768]
    bq = -(Wqf @ wh_bias)
    bk = -(Wkf @ wh_bias)
    bv = -(Wvf @ wh_bias)
    qkvb = np.stack([stripe_b(bq), stripe_b(bk), np.pad(bv, (0, 64))], 1)  # [128, 3]

    # head-striped proj weights [128, D]: rows 32h:32h+16 = head h, zeros
    # elsewhere (attnT garbage rows are the constant Z/Z = 1 there)
    projwS = np.zeros((128, D), np.float64)
    for h in range(H):
        projwS[32 * h : 32 * h + HS] = proj_W[:, h * HS : (h + 1) * HS].T

    # fold proj into the gate and expert W1 weights (fp64 host math)
    wgf = projwS @ w_gate.astype(np.float64)               # [128, E]
    cb = proj_b.astype(np.float64) @ w_gate.astype(np.float64)   # [E]
    w1f = np.einsum(
        "pd,ehd->eph", projwS, exp_W1.astype(np.float64)
    )                                                      # [E, 128, 600]
    b1f = exp_W1.astype(np.float64) @ proj_b.astype(np.float64) + exp_b1  # [E, 600]

    # b1 bias slices [128, E*5]: column e*5+mc holds b1f[e, h0:h0+hs]
    b1s = np.zeros((128, E * 5), np.float32)
    for e in range(E):
        for mc, (h0, hs) in enumerate(MCH):
            b1s[:hs, e * 5 + mc] = b1f[e, h0 : h0 + hs]

    f16 = np.float16
    common = {
        "wqkv": np.ascontiguousarray(wqkv.T).astype(f16),
        "qkvb": np.ascontiguousarray(qkvb).astype(np.float32),
        "wgf": np.ascontiguousarray(wgf).astype(f16),
        "cb": np.ascontiguousarray(cb[:, None]).astype(np.float32),
        "idm": np.eye(128, dtype=f16),
        "w1t": np.ascontiguousarray(w1f).astype(f16),
        "w2t": np.ascontiguousarray(exp_W2.transpose(0, 2, 1)).astype(f16),
        "b1s": b1s.astype(np.float32),
        "b2s": np.ascontiguousarray(exp_b2[:, None, :]).astype(f16),
    }
    in_maps = []
    for c in range(NCORES):
        xc = x[c * BL : (c + 1) * BL]                  # [8, 300, 768]
        xt = np.ascontiguousarray(
            xc.transpose(2, 0, 1).reshape(DIN, NTOK)
        ).astype(f16)
        in_maps.append({**common, "xt": xt})
    return in_maps


def kernel(x, wh_bias, wh_W, Wq, Wk, Wv, proj_W, proj_b,
           exp_W1, exp_b1, exp_W2, exp_b2, w_gate,
           debug=False, max_phase=9):
    global LAST_RESULT
    x = np.asarray(x, np.float32)
    wh_bias = np.asarray(wh_bias, np.float32)
    wh_W = np.asarray(wh_W, np.float32)
    Wq, Wk, Wv = (np.asarray(w, np.float32) for w in (Wq, Wk, Wv))
    proj_W = np.asarray(proj_W, np.float32)
    proj_b = np.asarray(proj_b, np.float32)
    exp_W1 = np.asarray(exp_W1, np.float32)
    exp_b1 = np.asarray(exp_b1, np.float32)
    exp_W2 = np.asarray(exp_W2, np.float32)
    exp_b2 = np.asarray(exp_b2, np.float32)
    w_gate = np.asarray(w_gate, np.float32)

    b2_nonzero = bool(np.any(exp_b2))
    in_maps = _prep_inputs(x, wh_bias, wh_W, Wq, Wk, Wv, proj_W, proj_b,
                           exp_W1, exp_b1, exp_W2, exp_b2, w_gate)

    nc = _get_module(b2_nonzero, debug=debug, max_phase=max_phase)
    for alloc in nc.m.functions[0].allocations:
        if isinstance(alloc, mybir.MemoryLocationSet) and alloc.kind == "ExternalInput":
            nm = alloc.memorylocations[0].name
            if nm not in in_maps[0]:
                continue  # partition_id etc., supplied by the runner
            got = in_maps[0][nm]
            assert tuple(got.shape) == tuple(alloc.tensor_shape), (
                nm, got.shape, alloc.tensor_shape)
            assert got.dtype == mybir.dt.np(alloc.dtype), (nm, got.dtype)
    res = run_bass_kernel_spmd(nc, in_maps, core_ids=list(range(NCORES)))
    LAST_RESULT = res
    out = np.stack([r["out"] for r in res.results])    # [8, 2400, 300]
    return out.reshape(B, T, D)


# revision 31
# speedup vs baseline: 1.7638x; 1.0164x over previous
"""Trainium2 Bass kernel for nn_MoEAdaptorLayer (whiten -> causal MHA -> proj
-> noisy-top-k gating (eval) -> 8 dense experts -> gated mixture * expert sum).

Sharding: data-parallel over batch. 64 batches -> 8 per core, params replicated.

v2 redesign vs baseline:
- fp16 everywhere on the matmul path (fp32 PSUM accumulation); halves DMA and
  removes the fp32r small-free-dim PE penalties.
- qkv: x shipped as one [768, 2400] fp16 tensor per core (6 big DMAs), matmuls
  in [128, 480] chunks.
- attention: single-orientation flash-style pass. Scores computed [s, t] only;
  exp on scalar; causal mask by multiplying the diagonal blocks with a
  triangular constant (split vector/gpsimd). AV and the softmax denominator
  come from ONE matmul per (head, s-chunk): stationary [v_h | ones] (32 cols)
  so the psum holds [av_h (16 rows) | Z_h replicated (16 rows)] per 32-stripe.
  One reciprocal + one fused min-mult STT per batch produce attnT directly in
  head-striped [128, tok] layout (no Z-pass, no output transposes).
- proj: stationary is zero-padded to the 32-stripe layout; 15 fp16 matmuls.
- gating: logits computed transposed ([8, tok], weight-stationary, 15 matmuls
  instead of 57 LDW-bound ones), PE-transposed back in 19 tiny transposes.
- experts: h chunked {128,128,128,128,88} (M=128-aligned LDWEIGHTS is ~2x
  faster than M=120), fp16 weights/activations.
"""

from contextlib import ExitStack

import numpy as np

import concourse.bass as bass
import concourse.tile as tile
import concourse.mybir as mybir
from concourse.bass_utils import run_bass_kernel_spmd

F16 = mybir.dt.float16
F32 = mybir.dt.float32
AX = mybir.AxisListType
OP = mybir.AluOpType
AF = mybir.ActivationFunctionType

B, T, DIN, D, E, H, HS = 64, 300, 768, 300, 8, 4, 16
NCORES = 8
BL = B // NCORES          # 8 batches per core
NTOK = BL * T             # 2400 tokens per core
DHK = H * HS              # 64
DH = 2 * D                # 600
TOPK = E // 2

K768 = [(i * 128, 128) for i in range(6)]
DC = [(0, 128), (128, 128), (256, 44)]          # 300 = 128+128+44
MCH = [(0, 128), (128, 128), (256, 128), (384, 128), (512, 88)]  # 600
TB = [(0, 128), (128, 128), (256, 44)]          # 300 tokens per batch
TOK = [(i * 128, 128) for i in range(18)] + [(2304, 96)]   # 2400 tokens
N5 = [(i * 480, 480) for i in range(5)]         # 2400 free-dim chunks
NEG = -1.0e30
RCLAMP = 16.0   # cap on 1/Z; keeps garbage stripe rows finite in fp16

_MAX_DRAIN_WAITS = 1
_WAIT_LIMIT = 1


def _split_waits(nc):
    """Walrus in this build caps sync waits per instruction; hoist excess
    waits onto same-engine NOPs inserted just before the instruction."""
    n = 0
    for f in nc.m.functions:
        for blk in f.blocks:
            insts = blk.instructions
            out = []
            changed = False
            for inst in insts:
                si = inst.sync_info
                waits = list(si.on_wait or []) if si is not None else []
                if len(waits) > _WAIT_LIMIT:
                    head, tail = waits[:-_WAIT_LIMIT], waits[-_WAIT_LIMIT:]
                    for i in range(0, len(head), _WAIT_LIMIT):
                        n += 1
                        nop = mybir.InstNoOp(name=f"waitnop{n}", ins=[], outs=[])
                        nop.engine = inst.engine
                        nop.sync_info = mybir.SyncInfo(
                            on_wait=head[i : i + _WAIT_LIMIT], on_update=[]
                        )
                        out.append(nop)
                    si.on_wait = tail
                    inst.sync_info = si
                    changed = True
                out.append(inst)
            if changed:
                blk.instructions = out


def _install_drain_patch():
    """This walrus build rejects CTRL instructions with more than a few sync
    waits; Tile's tail drain waits on every engine/DMA semaphore at once.
    Split the waits across a chain of single-wait drains."""
    if getattr(tile.TileContext, "_drain_patched", False):
        return

    def _patched(self, tick_clock, wait_clock):
        nc = self.nc
        drain_inst = nc.sync.drain()
        wait_clock.add_sem_waits(
            drain_inst.ins, tile.ScopedClock({None: tick_clock.global_clock})
        )
        ri = drain_inst.ins
        si = ri.sync_info
        waits = list(si.on_wait or []) if si is not None else []
        if len(waits) > _MAX_DRAIN_WAITS:
            si.on_wait = waits[:_MAX_DRAIN_WAITS]
            ri.sync_info = si
            for i in range(_MAX_DRAIN_WAITS, len(waits), _MAX_DRAIN_WAITS):
                d2 = nc.sync.drain()
                d2.ins.sync_info = mybir.SyncInfo(
                    on_wait=waits[i : i + _MAX_DRAIN_WAITS], on_update=[]
                )
        nc.all_engine_barrier()
        assert self.sems is not None
        popped = nc._tile_sem_poison_stack.pop()
        assert popped is self._sem_poison
        nc.clear_and_free_semaphores(list(self.sems.allocated().values()))
        nc.all_engine_barrier()

    tile.TileContext._drain_and_barrier = _patched
    tile.TileContext._drain_patched = True


def _build_module(b2_nonzero: bool, debug: bool = False, max_phase: int = 9):
    _install_drain_patch()
    nc = bass.Bass("TRN2", target_bir_lowering=False, debug=False)

    # ---- DRAM I/O ----
    xt_d = nc.dram_tensor("xt", [DIN, NTOK], F16, kind="ExternalInput")
    wqkv_d = nc.dram_tensor("wqkv", [DIN, 320], F16, kind="ExternalInput")
    qkvb_d = nc.dram_tensor("qkvb", [128, 3], F32, kind="ExternalInput")
    wgf_d = nc.dram_tensor("wgf", [128, E], F16, kind="ExternalInput")
    cb_d = nc.dram_tensor("cb", [E, 1], F32, kind="ExternalInput")
    w1_d = nc.dram_tensor("w1t", [E, 128, DH], F16, kind="ExternalInput")
    w2_d = nc.dram_tensor("w2t", [E, DH, D], F16, kind="ExternalInput")
    b1_d = nc.dram_tensor("b1s", [128, E * 5], F32, kind="ExternalInput")
    b2_d = nc.dram_tensor("b2s", [E, 1, D], F16, kind="ExternalInput")
    idm_d = nc.dram_tensor("idm", [128, 128], F16, kind="ExternalInput")
    out_d = nc.dram_tensor("out", [NTOK, D], F32, kind="ExternalOutput")
    if debug:
        dbg_q = nc.dram_tensor("dbg_q", [128, NTOK], F32, kind="ExternalOutput")
        dbg_k = nc.dram_tensor("dbg_k", [128, NTOK], F32, kind="ExternalOutput")
        dbg_at = nc.dram_tensor("dbg_at", [128, NTOK], F32, kind="ExternalOutput")
        dbg_xa = nc.dram_tensor("dbg_xa", [D, NTOK], F32, kind="ExternalOutput")
        dbg_gp = nc.dram_tensor("dbg_gp", [128, len(TOK) * E], F32, kind="ExternalOutput")

    with tile.TileContext(nc) as tc:
        with (
            tc.tile_pool(name="const", bufs=1) as cpool,
            tc.tile_pool(name="big", bufs=1) as bpool,
        ):
            # ---- persistent constants ----
            ident = cpool.tile([128, 128], F16)
            wqkv_s = []
            for kc, (k0, ks) in enumerate(K768):
                t = cpool.tile([128, 320], F16, tag=f"wqkv{kc}")
                nc.sync.dma_start(t[:, :], wqkv_d[k0 : k0 + ks, :])
                wqkv_s.append(t)
            qkvb_s = cpool.tile([128, 3], F32, tag="qkvbs")
            nc.sync.dma_start(qkvb_s[:, :], qkvb_d[:, :])
            wgf_s = cpool.tile([128, E], F16, tag="wgf")
            nc.sync.dma_start(wgf_s[:, :], wgf_d[:, :])
            cb_s = cpool.tile([E, 1], F32, tag="cb")
            nc.sync.dma_start(cb_s[:, :], cb_d[:, :])
            b1_s = cpool.tile([128, E * 5], F32, tag="b1s")
            nc.sync.dma_start(b1_s[:, :], b1_d[:, :])
            nc.sync.dma_start(ident[:, :], idm_d[:, :])

            # prefetch experts 0-3 weights up front on the gpsimd DMA queue
            w1_all, w2_all, b2_all = [], [], []
            for e in range(E // 2):
                t = bpool.tile([128, DH], F16, tag=f"w1_{e}", name=f"w1_{e}")
                nc.gpsimd.dma_start(t[:, :], w1_d[e, :, :])
                w1_all.append(t)
                w2t = []
                for mc, (h0, hs) in enumerate(MCH):
                    t = bpool.tile([hs, D], F16, tag=f"w2_{e}_{mc}", name=f"w2_{e}_{mc}")
                    nc.gpsimd.dma_start(t[:, :], w2_d[e, h0 : h0 + hs, :])
                    w2t.append(t)
                w2_all.append(w2t)
                if b2_nonzero:
                    t = bpool.tile([1, D], F16, tag=f"b2row{e}", name=f"b2row{e}")
                    nc.gpsimd.dma_start(t[:, :], b2_d[e, :, :])
                    b2_all.append(t)

            # ---- activations ----
            _mid = ExitStack()
            mpool = _mid.enter_context(tc.tile_pool(name="mid", bufs=1))
            # heads striped at 32-partition bases (rows 32h:32h+16 per head)
            qTs = mpool.tile([128, NTOK], F16, tag="qTs")
            kTs = mpool.tile([128, NTOK], F16, tag="kTs")
            qTb = mpool.tile([DHK, NTOK], F16, tag="qTb")
            kTb = mpool.tile([DHK, NTOK], F16, tag="kTb")
            vT = mpool.tile([DHK, NTOK], F16, tag="vT")
            # v_s[sc]: [ss, b, h, 64] = [v_h(16) | 0(16) | ones(16) | 0(16)];
            # the av/Z layout this produces keeps every vector-op partition
            # base 32-aligned
            v_s = [
                mpool.tile([128, BL, H, 64], F16, tag=f"v{sc}", name=f"v{sc}")
                for sc in range(3)
            ]
            # attnT [128, NTOK]: rows 32h:32h+16 = head h, other rows are
            # the constant 1.0 (Z/Z); folded weights are zero there
            attnT = bpool.tile([128, NTOK], F16, tag="attnT")
            # v_s constant columns [v16 | ones48]: one matmul per (h, sc)
            # produces [av (16 rows) | Z replicated (48 rows)]
            for sc in range(3):
                nc.gpsimd.memset(v_s[sc][:, :, :, 16:64], 1.0)

            # ================= phase 1: qkv =================
            with (
                tc.tile_pool(name="xt", bufs=1) as xpool,
                tc.tile_pool(name="ps1", bufs=4, space="PSUM") as ps1,
            ):
                xt_s = [
                    xpool.tile([128, NTOK], F16, tag=f"xt{kc}", name=f"xt{kc}")
                    for kc in range(6)
                ]
                for half in range(2):
                    hlo, hhi = half * 1200, (half + 1) * 1200
                    for kc, (k0, ks) in enumerate(K768):
                        nc.scalar.dma_start(
                            xt_s[kc][:, hlo:hhi], xt_d[k0 : k0 + ks, hlo:hhi]
                        )
                specs = ((qTs, 128, 0), (kTs, 128, 128), (vT, DHK, 256))
                for mi, (dst, mw, c0) in enumerate(specs):
                    for n0, ns in N5:
                        ps = ps1.tile([128, 480], F32, tag="qkvp")
                        for kc in range(6):
                            nc.tensor.matmul(
                                ps[:mw, :ns],
                                wqkv_s[kc][:, c0 : c0 + mw],
                                xt_s[kc][:, n0 : n0 + ns],
                                start=(kc == 0), stop=(kc == 5),
                            )
                        nc.scalar.activation(
                            dst[:, n0 : n0 + ns], ps[:mw, :ns], AF.Identity,
                            bias=qkvb_s[:mw, mi : mi + 1], scale=1.0,
                        )
                # move heads 2-3 down to base 0/32 (DMA shifts partitions)
                nc.sync.dma_start(qTb[:, :], qTs[64:128, :])
                nc.sync.dma_start(kTb[:, :], kTs[64:128, :])
                if debug:
                    dq = mpool.tile([128, NTOK], F32, tag="dbgq")
                    nc.vector.tensor_copy(dq[:, :], qTs[:, :])
                    nc.sync.dma_start(dbg_q[:, :], dq[:, :])
                    dk = mpool.tile([128, NTOK], F32, tag="dbgk")
                    nc.vector.tensor_copy(dk[:, :], kTs[:, :])
                    nc.sync.dma_start(dbg_k[:, :], dk[:, :])

            # ================= phase 2: attention =================
            if max_phase >= 2:
             with (
                tc.tile_pool(name="att", bufs=2) as apool,
                tc.tile_pool(name="ps2", bufs=2, space="PSUM") as ps2,
                tc.tile_pool(name="ps2z", bufs=2, space="PSUM") as ps2z,
            ):
                for b in range(BL):
                    base = b * T
                    # v = vT.T per s-chunk (all heads at once)
                    for sc, (s0, ss) in enumerate(TB):
                        vtr = ps2.tile([128, DHK], F16, tag="vtr")
                        nc.tensor.transpose(
                            vtr[:ss, :], vT[:, base + s0 : base + s0 + ss],
                            ident[0:DHK, 0:DHK],
                        )
                        nc.scalar.copy(
                            v_s[sc][0:ss, b, :, 0:16],
                            vtr[:ss, :].rearrange("p (h k) -> p h k", h=H),
                        )
                    # FT pass: scores [s, t] with the causal -30000 mask added
                    # into the diag block via an identity-stationary matmul;
                    # exp; then one matmul per (h, sc) computing
                    # [av_h | 0 | Z_h | 0] in a 64-row block; heads 2g / 2g+1
                    # at row bases 0 / 64 of azb[g]
                    azb = [
                        ps2z.tile([128, T], F32, tag=f"azb{g}", name=f"azb{g}")
                        for g in range(2)
                    ]
                    ft = {}
                    for h in range(H):
                        qt = qTs if h < 2 else qTb
                        kt = kTs if h < 2 else kTb
                        q0 = k0 = (h % 2) * 32
                        for sc, (s0, ss) in enumerate(TB):
                            tlen = T - s0
                            weiT = ps2.tile([128, T], F32, tag="weiT")
                            nc.tensor.matmul(
                                weiT[:ss, :tlen],
                                kt[k0 : k0 + HS, base + s0 : base + s0 + ss],
                                qt[q0 : q0 + HS, base + s0 : base + T],
                                start=True, stop=True,
                            )
                            f = apool.tile([128, T], F16, tag=f"ft{h}{sc}")
                            ft[(h, sc)] = f
                            nc.scalar.activation(f[:ss, :tlen], weiT[:ss, :tlen], AF.Exp)
                            nc.gpsimd.affine_select(
                                out=f[:ss, :ss], in_=f[:ss, :ss],
                                compare_op=OP.is_ge, fill=0.0,
                                base=0, channel_multiplier=-1, pattern=[[1, ss]],
                            )
                        for sc, (s0, ss) in enumerate(TB):
                            nc.tensor.matmul(
                                azb[h // 2][64 * (h % 2) : 64 * (h % 2) + 64, s0:T],
                                v_s[sc][0:ss, b, h, :],
                                ft[(h, sc)][0:ss, 0 : T - s0],
                                start=(sc == 0), stop=(sc == 2),
                            )
                    # 1/azb on the scalar table engine (cheap); only the
                    # Z-recip rows are ever read back, so the garbage
                    # av-recip rows (possibly inf) are harmless
                    rz = [
                        apool.tile([128, T], F32, tag=f"rz{g}", name=f"rz{g}")
                        for g in range(2)
                    ]
                    for g in range(2):
                        nc.vector.reciprocal(rz[g][:, :], azb[g][:, :])
                    for h in range(H):
                        g, o = h // 2, 64 * (h % 2)
                        nc.vector.scalar_tensor_tensor(
                            attnT[32 * h : 32 * h + 32, base : base + T],
                            rz[g][o + 32 : o + 64, :], RCLAMP,
                            azb[g][o : o + 32, :],
                            op0=OP.min, op1=OP.mult,
                        )
                if debug:
                    da = apool.tile([128, NTOK], F32, tag="dbga")
                    nc.vector.tensor_copy(da[:, :], attnT[:, :])
                    nc.sync.dma_start(dbg_at[:, :], da[:, :])

            _mid.close()
            _late = ExitStack()
            lpool = _late.enter_context(tc.tile_pool(name="late", bufs=1))
            gp = lpool.tile([128, len(TOK) * E], F32, tag="gp")
            G = lpool.tile([128, len(TOK) * D], F32, tag="G")
            S = lpool.tile([128, len(TOK) * D], F32, tag="S")

            # ================= phase 4: gating =================
            if max_phase >= 4:
             with (
                tc.tile_pool(name="gat", bufs=1) as gpool,
                tc.tile_pool(name="ps4", bufs=2, space="PSUM") as ps4,
                tc.tile_pool(name="ps4t", bufs=1, space="PSUM") as ps4t,
             ):
                NT_ = len(TOK)
                # logits transposed: [8, tok], folded proj+gate stationary
                logT = gpool.tile([8, NTOK], F16, tag="logT")
                for n0, ns in N5:
                    lg = ps4.tile([8, 480], F32, tag="lg")
                    nc.tensor.matmul(
                        lg[:, :ns], wgf_s[:, :], attnT[:, n0 : n0 + ns],
                        start=True, stop=True,
                    )
                    nc.scalar.activation(
                        logT[:, n0 : n0 + ns], lg[:, :ns], AF.Identity,
                        bias=cb_s[:, :], scale=1.0,
                    )
                # transpose back to [tok, 8] in one psum bank
                ltp = ps4t.tile([128, NT_, E], F16, tag="ltp")
                for ti, (t0, ts_) in enumerate(TOK):
                    nc.tensor.transpose(
                        ltp[:ts_, ti, :], logT[:, t0 : t0 + ts_], ident[0:8, 0:8]
                    )
                L3 = gpool.tile([128, NT_, E], F32, tag="L3")
                nc.scalar.copy(L3[:, :, :], ltp[:, :, :])
                work = gpool.tile([128, NT_, E], F32, tag="work")
                nc.vector.tensor_copy(work[:, :, :], L3[:, :, :])
                m = gpool.tile([128, NT_, 4], F32, tag="m")
                eqm = gpool.tile([128, NT_, E], F32, tag="eqm")
                for r in range(TOPK):
                    nc.vector.tensor_reduce(
                        m[:, :, r : r + 1], work[:, :, :], axis=AX.X, op=OP.max,
                        opt_input=False, opt_output=False,
                    )
                    if r < TOPK - 1:
                        nc.vector.tensor_tensor(
                            eqm[:, :, :], work[:, :, :],
                            m[:, :, r : r + 1].broadcast_to((128, NT_, E)),
                            op=OP.is_equal,
                        )
                        nc.vector.scalar_tensor_tensor(
                            work[:, :, :], eqm[:, :, :], NEG, work[:, :, :],
                            op0=OP.mult, op1=OP.add,
                        )
                sel = gpool.tile([128, NT_, E], F32, tag="sel")
                nc.vector.tensor_tensor(
                    sel[:, :, :], L3[:, :, :],
                    m[:, :, 3:4].broadcast_to((128, NT_, E)), op=OP.is_ge,
                )
                el = gpool.tile([128, NT_, E], F32, tag="el")
                nc.scalar.activation(el[:, :, :], L3[:, :, :], AF.Exp)
                elm = gpool.tile([128, NT_, E], F32, tag="elm")
                nc.vector.tensor_tensor(elm[:, :, :], el[:, :, :], sel[:, :, :], op=OP.mult)
                zg = gpool.tile([128, NT_, 1], F32, tag="zg")
                nc.vector.tensor_reduce(
                    zg[:, :, :], elm[:, :, :], axis=AX.X, op=OP.add,
                    opt_input=False, opt_output=False,
                )
                rzg = gpool.tile([128, NT_, 1], F32, tag="rzg")
                nc.vector.reciprocal(rzg[:, :, :], zg[:, :, :])
                nc.vector.tensor_tensor(
                    gp[:, :].rearrange("p (t e) -> p t e", e=E), elm[:, :, :],
                    rzg[:, :, :].broadcast_to((128, NT_, E)),
                    op=OP.mult,
                )
            if debug and max_phase >= 4:
                nc.sync.dma_start(dbg_gp[:, :], gp[:, :])

            # ================= phase 5: experts =================
            if max_phase >= 5:
             with (
                tc.tile_pool(name="outp", bufs=4) as opool,
                tc.tile_pool(name="wexp", bufs=1) as wpool,
                tc.tile_pool(name="ht", bufs=2) as hpool,
                tc.tile_pool(name="ps5", bufs=4, space="PSUM") as ps5h,
                tc.tile_pool(name="ps5b", bufs=4, space="PSUM") as ps5e,
            ):
                for e in range(E // 2, E):
                    t = wpool.tile([128, DH], F16, tag=f"w1_{e}", name=f"w1_{e}")
                    nc.gpsimd.dma_start(t[:, :], w1_d[e, :, :])
                    w1_all.append(t)
                    w2t = []
                    for mc, (h0, hs) in enumerate(MCH):
                        t = wpool.tile([hs, D], F16, tag=f"w2_{e}_{mc}", name=f"w2_{e}_{mc}")
                        nc.gpsimd.dma_start(t[:, :], w2_d[e, h0 : h0 + hs, :])
                        w2t.append(t)
                    w2_all.append(w2t)
                    if b2_nonzero:
                        t = wpool.tile([1, D], F16, tag=f"b2row{e}", name=f"b2row{e}")
                        nc.gpsimd.dma_start(t[:, :], b2_d[e, :, :])
                        b2_all.append(t)
                ones = None
                if b2_nonzero:
                    ones = cpool.tile([1, NTOK], F16, tag="ones")
                    nc.gpsimd.memset(ones[:, :], 1.0)
                for e in range(E):
                    w1t = w1_all[e]
                    w2t = w2_all[e]
                    b2row = b2_all[e] if b2_nonzero else None
                    ht = []
                    for mc, (h0, hs) in enumerate(MCH):
                        t = hpool.tile([hs, NTOK], F16, tag=f"ht{mc}")
                        ht.append(t)
                        for n, (n0, ns) in enumerate(N5):
                            hp = ps5h.tile([128, 480], F32, tag="hp")
                            nc.tensor.matmul(
                                hp[:hs, :ns], w1t[:, h0 : h0 + hs],
                                attnT[:, n0 : n0 + ns],
                                start=True, stop=True,
                            )
                            nc.scalar.activation(
                                t[:, n0 : n0 + ns], hp[:hs, :ns], AF.Relu,
                                bias=b1_s[:hs, e * 5 + mc : e * 5 + mc + 1], scale=1.0,
                            )
                    for ti, (t0, ts_) in enumerate(TOK):
                        eo = ps5e.tile([128, D], F32, tag="eo")
                        for mc in range(5):
                            nc.tensor.matmul(
                                eo[:ts_, :], ht[mc][:, t0 : t0 + ts_], w2t[mc][:, :],
                                start=(mc == 0), stop=(mc == 4 and not b2_nonzero),
                            )
                        if b2_nonzero:
                            nc.tensor.matmul(
                                eo[:ts_, :], ones[:, t0 : t0 + ts_], b2row[:, :],
                                start=False, stop=True,
                            )
                        gsc = gp[:ts_, ti * E + e : ti * E + e + 1]
                        gsl = G[:ts_, ti * D : (ti + 1) * D]
                        ssl = S[:ts_, ti * D : (ti + 1) * D]
                        if e == 0:
                            nc.vector.tensor_scalar_mul(gsl, eo[:ts_, :], gsc)
                            nc.scalar.copy(ssl, eo[:ts_, :])
                        else:
                            nc.vector.scalar_tensor_tensor(
                                gsl, eo[:ts_, :], gsc, gsl, op0=OP.mult, op1=OP.add
                            )
                            nc.vector.tensor_tensor(ssl, eo[:ts_, :], ssl, op=OP.add)
                        if e == E - 1:
                            o = opool.tile([128, D], F32, tag="o")
                            nc.vector.tensor_tensor(o[:ts_, :], gsl, ssl, op=OP.mult)
                            nc.sync.dma_start(out_d[t0 : t0 + ts_, :], o[:ts_, :])

            _late.close()

    _split_waits(nc)
    return nc


_CACHE = {}
LAST_RESULT = None


def _get_module(b2_nonzero: bool, debug: bool = False, max_phase: int = 9):
    key = (b2_nonzero, debug, max_phase)
    if key not in _CACHE:
        _CACHE[key] = _build_module(b2_nonzero, debug=debug, max_phase=max_phase)
    return _CACHE[key]


def _prep_inputs(x, wh_bias, wh_W, Wq, Wk, Wv, proj_W, proj_b,
                 exp_W1, exp_b1, exp_W2, exp_b2, w_gate):
    # fold whiten into qkv; fold attention scale into q
    scale = float(D) ** -0.5
    Wqf = (Wq.reshape(DHK, D) @ wh_W) * scale          # [64, 768]
    Wkf = Wk.reshape(DHK, D) @ wh_W
    Wvf = Wv.reshape(DHK, D) @ wh_W

    def stripe(w):                                     # [64, 768] -> [128, 768]
        out = np.zeros((128, DIN), np.float32)
        for h in range(H):
            out[h * 32 : h * 32 + HS] = w[h * HS : (h + 1) * HS]
        return out

    def stripe_b(v):                                   # [64] -> [128]
        out = np.zeros(128, np.float32)
        for h in range(H):
            out[h * 32 : h * 32 + HS] = v[h * HS : (h + 1) * HS]
        return out

    wqkv = np.concatenate([stripe(Wqf), stripe(Wkf), Wvf], 0)   # [320, 768]
    bq = -(Wqf @ wh_bias)
    bk = -(Wkf @ wh_bias)
    bv = -(Wvf @ wh_bias)
    qkvb = np.stack([stripe_b(bq), stripe_b(bk), np.pad(bv, (0, 64))], 1)  # [128, 3]

    # head-striped proj weights [128, D]: rows 32h:32h+16 = head h, zeros
    # elsewhere (attnT garbage rows are the constant Z/Z = 1 there)
    projwS = np.zeros((128, D), np.float64)
    for h in range(H):
        projwS[32 * h : 32 * h + HS] = proj_W[:, h * HS : (h + 1) * HS].T

    # fold proj into the gate and expert W1 weights (fp64 host math)
    wgf = projwS @ w_gate.astype(np.float64)               # [128, E]
    cb = proj_b.astype(np.float64) @ w_gate.astype(np.float64)   # [E]
    w1f = np.einsum(
        "pd,ehd->eph", projwS, exp_W1.astype(np.float64)
    )                                                      # [E, 128, 600]
    b1f = exp_W1.astype(np.float64) @ proj_b.astype(np.float64) + exp_b1  # [E, 600]

    # b1 bias slices [128, E*5]: column e*5+mc holds b1f[e, h0:h0+hs]
    b1s = np.zeros((128, E * 5), np.float32)
    for e in range(E):
        for mc, (h0, hs) in enumerate(MCH):
            b1s[:hs, e * 5 + mc] = b1f[e, h0 : h0 + hs]

    f16 = np.float16
    common = {
        "wqkv": np.ascontiguousarray(wqkv.T).astype(f16),
        "qkvb": np.ascontiguousarray(qkvb).astype(np.float32),
        "wgf": np.ascontiguousarray(wgf).astype(f16),
        "cb": np.ascontiguousarray(cb[:, None]).astype(np.float32),
        "idm": np.eye(128, dtype=f16),
        "w1t": np.ascontiguousarray(w1f).astype(f16),
        "w2t": np.ascontiguousarray(exp_W2.transpose(0, 2, 1)).astype(f16),
        "b1s": b1s.astype(np.float32),
        "b2s": np.ascontiguousarray(exp_b2[:, None, :]).astype(f16),
    }
    in_maps = []
    for c in range(NCORES):
        xc = x[c * BL : (c + 1) * BL]                  # [8, 300, 768]
        xt = np.ascontiguousarray(
            xc.transpose(2, 0, 1).reshape(DIN, NTOK)
        ).astype(f16)
        in_maps.append({**common, "xt": xt})
    return in_maps


def kernel(x, wh_bias, wh_W, Wq, Wk, Wv, proj_W, proj_b,
           exp_W1, exp_b1, exp_W2, exp_b2, w_gate,
           debug=False, max_phase=9):
    global LAST_RESULT
    x = np.asarray(x, np.float32)
    wh_bias = np.asarray(wh_bias, np.float32)
    wh_W = np.asarray(wh_W, np.float32)
    Wq, Wk, Wv = (np.asarray(w, np.float32) for w in (Wq, Wk, Wv))
    proj_W = np.asarray(proj_W, np.float32)
    proj_b = np.asarray(proj_b, np.float32)
    exp_W1 = np.asarray(exp_W1, np.float32)
    exp_b1 = np.asarray(exp_b1, np.float32)
    exp_W2 = np.asarray(exp_W2, np.float32)
    exp_b2 = np.asarray(exp_b2, np.float32)
    w_gate = np.asarray(w_gate, np.float32)

    b2_nonzero = bool(np.any(exp_b2))
    in_maps = _prep_inputs(x, wh_bias, wh_W, Wq, Wk, Wv, proj_W, proj_b,
                           exp_W1, exp_b1, exp_W2, exp_b2, w_gate)

    nc = _get_module(b2_nonzero, debug=debug, max_phase=max_phase)
    for alloc in nc.m.functions[0].allocations:
        if isinstance(alloc, mybir.MemoryLocationSet) and alloc.kind == "ExternalInput":
            nm = alloc.memorylocations[0].name
            if nm not in in_maps[0]:
                continue  # partition_id etc., supplied by the runner
            got = in_maps[0][nm]
            assert tuple(got.shape) == tuple(alloc.tensor_shape), (
                nm, got.shape, alloc.tensor_shape)
            assert got.dtype == mybir.dt.np(alloc.dtype), (nm, got.dtype)
    res = run_bass_kernel_spmd(nc, in_maps, core_ids=list(range(NCORES)))
    LAST_RESULT = res
    out = np.stack([r["out"] for r in res.results])    # [8, 2400, 300]
    return out.reshape(B, T, D)
